# revision 15
# baseline (speedup 1.0000x reference)
"""GCN encoder (2x GCNConv + BatchNorm/ReLU) on 8 Trainium2 NeuronCores.

Math: with s = 1/sqrt(deg+1) (deg = in-degree by dst), the GCN edge norm
factorizes: norm_e = s[src]*s[dst], so for any node features H,
    A(H) := segsum(norm_e * H[src], dst) + H * s^2
          = s * ( segsum( (s*H)[src], dst) + (s*H) )
and GCNConv(H, W, b) = A(H)@W + b = A(H@W) + b, so the whole net needs only
TWO sparse aggregations (layer1 on (s*x)@W1, layer2 on s*post-BN hidden),
and mu / log_std share the second one.

v2 design (vs the AllGather-everything baseline):
  * Stage 1 is REPLICATED: every core streams the full (s-prescaled, bf16)
    x^T from its local DRAM and computes the whole 50176-row message table
    with the otherwise-idle PE -> no AllGather #1 (saves a ~250us collective).
  * Message tables are bf16 (halves collective + HBM gather traffic).
  * Segment sums run on the PE: the gather layout puts slot-aligned edges on
    partitions, so each 128-column slot is accumulated into a per-block PSUM
    bank with an identity-matmul (PSUM fp32 accumulate) -- DVE only does one
    scale per block.  Padding slots point at guaranteed-zero table rows.
  * Gather calls carry up to 32*128 = 4096 indices (SWDGE ring allows
    ndesc = n/16+1 <= 1024), amortizing the ~1us fixed descriptor-gen cost.
  * Self-loop term is added from an SBUF-resident copy of the core's own
    table slab (recomputed from a small per-core x slice) via one more
    identity-matmul per block.
  * BN batch stats go through a tiny AllGather (cheaper than AllReduce) and
    are combined locally.  Only layer 2 still needs a real table AllGather.

Host preprocessing (free): integer edge plan, permutations, index tensors,
plus input marshaling: x is row-scaled by s, permuted to table order, and
cast to bf16 ((diag(s) x) @ W1 == s * (x @ W1)).  All O(N*D^2) and per-edge
float math stays on device.
"""

import numpy as np

N_NODES = 50000
N_EDGES = 800000
D_IN = 128
D_HID = 128
D_LAT = 64
BN_EPS = 1e-5
N_CORES = 8
P = 128
LO_CORES = 5  # cores 0..4 form the "lo" table half; 5*6272=31360 < 32768
              # (dma_gather int16 indices address at most 32768 rows per call)

CALL_COLS = 7      # gather call size: 7 cols * 128 = 896 idxs (hw ring cap)
NUM_Q = 4          # SWDGE queues
XCHUNK_BLOCKS = 16  # stage-1 x^T streaming chunk (16 blocks = 2048 cols)

_CACHE = {}


# ----------------------------------------------------------------------------
# Host-side preprocessing
# ----------------------------------------------------------------------------


def _wrap_idx(lin):
    """dma_gather idx layout: position i -> [i%16, i//16], replicated to 128
    partitions. lin: [n] int array (n % 16 == 0) -> [128, n//16] int16."""
    n = lin.shape[0]
    w = lin.reshape(n // 16, 16).T.astype(np.int16)  # [16, n//16]
    return np.tile(w, (8, 1))


def _pack_calls(D, call_cols):
    """Slice the global column space into calls of <= call_cols columns.

    A call may cover partial blocks; each call carries its piece list
    [(block, col_off_in_call, width, first, last)].
    """
    C0 = np.concatenate([[0], np.cumsum(D)]).astype(np.int64)
    ct = int(C0[-1])
    calls = []
    for c0 in range(0, ct, call_cols):
        c1 = min(c0 + call_cols, ct)
        pieces = []
        for b in range(len(D)):
            lo = max(c0, int(C0[b]))
            hi = min(c1, int(C0[b + 1]))
            if lo < hi:
                pieces.append(
                    (b, lo - c0, hi - lo, lo == int(C0[b]), hi == int(C0[b + 1]))
                )
        calls.append((c0, c1 - c0, tuple(pieces)))
    return C0, calls


def _build_pass(tcoord_src, tkey_dst, n_cores, npc, blocks, call_cols,
                pad_idx, idx_base):
    """Build one gather pass layout.

    tcoord_src: per-edge source table coord (already offset for hi pass)
    tkey_dst:   per-edge dst node key in THIS pass's permutation
    Returns D [blocks], C0, calls, idx arrays [n_cores, 128, c_total] int32.
    """
    deg = np.bincount(tkey_dst, minlength=n_cores * npc)
    d3 = deg.reshape(n_cores, blocks, P)
    D = d3.max(axis=(0, 2)).astype(np.int64)
    D = np.maximum(D, 1)
    C0, calls = _pack_calls(D, call_cols)
    c_total = int(C0[-1])

    idx = np.full((n_cores, P, c_total), pad_idx - idx_base, dtype=np.int32)
    eorder = np.argsort(tkey_dst, kind="stable")
    k_s = tkey_dst[eorder]
    src_s = (tcoord_src[eorder] - idx_base).astype(np.int32)
    grp = np.searchsorted(k_s, k_s)
    slot = np.arange(k_s.size) - grp
    core_e = k_s // npc
    local_e = k_s % npc
    b_e = local_e // P
    p_e = local_e % P
    col_e = C0[b_e] + slot
    assert (slot < D[b_e]).all()
    idx[core_e, p_e, col_e] = src_s
    return D, C0, calls, idx, c_total


def _idx_to_wrapped(idx):
    """[n_cores, 128, c_total] int32 -> wrapped int16 [n_cores, 128, 8*c_total].

    Global linear position order is column-major (i = c*128 + p); contiguous
    position chunks map to contiguous wrapped columns, so any call covering
    cols [c0, c1) reads the wrapped slice [:, 8*c0 : 8*c1]."""
    n_cores, _, c_total = idx.shape
    out = np.empty((n_cores, 128, 8 * c_total), dtype=np.int16)
    for k in range(n_cores):
        lin = idx[k].T.reshape(-1)
        out[k] = _wrap_idx(lin)
    return out


def _plan(edge_index, n_nodes, n_cores, call_cols):
    src = np.asarray(edge_index[0], dtype=np.int64)
    dst = np.asarray(edge_index[1], dtype=np.int64)

    deg_in = np.bincount(dst, minlength=n_nodes).astype(np.int64)
    s = (1.0 / np.sqrt((deg_in + 1).astype(np.float64))).astype(np.float32)

    n_local = (n_nodes + n_cores - 1) // n_cores
    blocks = (n_local + 1 + P - 1) // P
    npc = blocks * P
    lo_rows = LO_CORES * npc
    assert lo_rows < 32768 and (n_cores * npc - lo_rows) < 32768

    # ---- core assignment: deal by total-degree rank (balances edge load and
    # aligns block-degree profiles across cores).
    order = np.argsort(-deg_in, kind="stable")
    rank_of = np.empty(n_nodes, dtype=np.int64)
    rank_of[order] = np.arange(n_nodes)
    core_of = rank_of % n_cores

    src_is_lo = core_of[src] < LO_CORES
    dlo = np.bincount(dst[src_is_lo], minlength=n_nodes)
    dhi = np.bincount(dst[~src_is_lo], minlength=n_nodes)

    # main layout: per-core locals sorted by lo-degree (tight LO padding)
    local_of = np.empty(n_nodes, dtype=np.int64)
    node2hi = np.empty(n_nodes, dtype=np.int64)
    for k in range(n_cores):
        nodes_k = np.nonzero(core_of == k)[0]
        o = nodes_k[np.argsort(-dlo[nodes_k], kind="stable")]
        local_of[o] = np.arange(o.size)
        o2 = nodes_k[np.argsort(-dhi[nodes_k], kind="stable")]
        node2hi[o2] = k * npc + np.arange(o2.size)
    node2table = core_of * npc + local_of


    # ---- gather-source row numbering: tables are [(core,part), (block,feat)]
    # 2-D tensors, so node (core k, local j=b*128+p) lives at flat row
    # (k*128+p)*blocks + b of its half (hi half: k-LO_CORES).
    def kpb_row(core, local, core0):
        return ((core - core0) * P + local % P) * blocks + local // P

    node2row = np.where(
        core_of < LO_CORES,
        kpb_row(core_of, local_of, 0),
        kpb_row(core_of, local_of, LO_CORES),
    )
    pad_lo = kpb_row(0, npc - 1, 0)
    pad_hi = kpb_row(N_CORES - 1, npc - 1, LO_CORES)

    # ---- LO pass on the main permutation
    D_lo, C0_lo, calls_lo, idx_lo, ct_lo = _build_pass(
        node2row[src[src_is_lo]], node2table[dst[src_is_lo]], n_cores, npc,
        blocks, call_cols, pad_lo, 0,
    )

    # ---- HI pass on the hi permutation
    D_hi, C0_hi, calls_hi, idx_hi, ct_hi = _build_pass(
        node2row[src[~src_is_lo]], node2hi[dst[~src_is_lo]], n_cores, npc,
        blocks, call_cols, pad_hi, 0,
    )

    # ---- combine map: main-layout local j gets acc_hi[himap[j]] added
    himap = np.full((n_cores, npc), npc - 1, dtype=np.int64)  # pad -> pad row
    for k in range(n_cores):
        nodes_k = np.nonzero(core_of == k)[0]
        himap[k, local_of[nodes_k]] = node2hi[nodes_k] % npc

    widx_lo = _idx_to_wrapped(idx_lo)
    widx_hi = _idx_to_wrapped(idx_hi)
    himap_row = (himap % P) * blocks + himap // P
    widx_cb = np.stack([_wrap_idx(himap_row[k]) for k in range(n_cores)])

    # per-core node lists and s in the MAIN layout
    node_of = np.full((n_cores, npc), -1, dtype=np.int64)
    s_arr = np.zeros((n_cores, P, blocks), dtype=np.float32)
    for k in range(n_cores):
        nodes_k = np.nonzero(core_of == k)[0]
        loc = local_of[nodes_k]
        node_of[k, loc] = nodes_k
        s_arr[k, loc % P, loc // P] = s[nodes_k]

    return dict(
        s=s,
        node2table=node2table,
        node_of=node_of,
        npc=npc,
        blocks=blocks,
        n_local=n_local,
        lo_rows=lo_rows,
        s_arr=s_arr,
        D_lo=D_lo, C0_lo=C0_lo, calls_lo=calls_lo, idx_lo=idx_lo, ct_lo=ct_lo,
        D_hi=D_hi, C0_hi=C0_hi, calls_hi=calls_hi, idx_hi=idx_hi, ct_hi=ct_hi,
        himap=himap,
        widx_lo=widx_lo, widx_hi=widx_hi, widx_cb=widx_cb,
    )


def _to_bf16(a):
    import ml_dtypes

    return a.astype(ml_dtypes.bfloat16)


def _host_inputs(plan, x, W1, Wmu, Wls, bmu, bls, gamma, beta):
    npc = plan["npc"]
    node_of = plan["node_of"]
    s = plan["s"]
    wcat = np.concatenate([Wmu, Wls], axis=1).astype(np.float32)
    bcat = np.concatenate([bmu, bls]).astype(np.float32).reshape(D_HID, 1)

    # full table-ordered, s-prescaled x^T (shared by all cores)
    xs_tab = np.zeros((N_CORES * npc, x.shape[1]), dtype=np.float32)
    for k in range(N_CORES):
        nodes = node_of[k]
        valid = nodes >= 0
        xs_tab[k * npc + np.nonzero(valid)[0]] = (
            x[nodes[valid]] * s[nodes[valid]][:, None]
        )
    xsT = np.ascontiguousarray(_to_bf16(xs_tab.T))  # [128, 8*npc] bf16

    per_core = []
    for k in range(N_CORES):
        per_core.append(
            {
                "xsT": xsT,
                "xsTo": np.ascontiguousarray(xsT[:, k * npc : (k + 1) * npc]),
                "W1": np.ascontiguousarray(_to_bf16(W1)),
                "Wcat": np.ascontiguousarray(_to_bf16(wcat)),
                "bcat": bcat,
                "s_arr": np.ascontiguousarray(plan["s_arr"][k]),
                "widx_lo": np.ascontiguousarray(plan["widx_lo"][k]),
                "widx_hi": np.ascontiguousarray(plan["widx_hi"][k]),
                "widx_cb": np.ascontiguousarray(plan["widx_cb"][k]),
                "gamma": gamma.astype(np.float32).reshape(D_HID, 1),
                "beta": beta.astype(np.float32).reshape(D_HID, 1),
            }
        )
    return per_core


def _postprocess(plan, outs):
    n_nodes = int(plan["node_of"].max()) + 1
    mu = np.zeros((n_nodes, D_LAT), dtype=np.float32)
    ls = np.zeros((n_nodes, D_LAT), dtype=np.float32)
    node_of = plan["node_of"]
    npc = node_of.shape[1]
    blocks = npc // 128
    for k in range(N_CORES):
        nodes = node_of[k]
        valid = nodes >= 0
        o = np.asarray(outs[k]).astype(np.float32).reshape(128, blocks, 128)
        o = o.transpose(1, 0, 2).reshape(npc, 128)  # node-major (b*128+p)
        mu[nodes[valid]] = o[valid.nonzero()[0], :D_LAT]
        ls[nodes[valid]] = o[valid.nonzero()[0], D_LAT:]
    return mu, ls


# ----------------------------------------------------------------------------
# Numpy emulation of the device program (fp32; bf16 rounding not modeled)
# ----------------------------------------------------------------------------


def _numpy_model(plan, per_core, x, W1, wcat, bcat, gamma, beta, n_real):
    npc, blocks = plan["npc"], plan["blocks"]
    lo_rows = plan["lo_rows"]
    node_of, s = plan["node_of"], plan["s"]

    table1 = np.zeros((N_CORES * npc, D_HID), dtype=np.float32)
    for k in range(N_CORES):
        nodes = node_of[k]
        valid = nodes >= 0
        table1[k * npc + np.nonzero(valid)[0]] = (
            (x[nodes[valid]] * s[nodes[valid]][:, None]) @ W1
        )

    def aggregate(table, k):
        out = np.zeros((npc, table.shape[1]), dtype=np.float32)
        mlo = table[:lo_rows][plan["idx_lo"][k]]  # [P, ct, F]
        C0 = plan["C0_lo"]
        for b in range(blocks):
            out[b * P:(b + 1) * P] += mlo[:, C0[b]:C0[b + 1], :].sum(axis=1)
        mhi = table[lo_rows:][plan["idx_hi"][k]]
        acc_hi = np.zeros((npc, table.shape[1]), dtype=np.float32)
        C0 = plan["C0_hi"]
        for b in range(blocks):
            acc_hi[b * P:(b + 1) * P] = mhi[:, C0[b]:C0[b + 1], :].sum(axis=1)
        out += acc_hi[plan["himap"][k]]
        own0 = k * npc
        sk = per_core[k]["s_arr"].T.reshape(-1, 1)
        return (out + table[own0:own0 + npc]) * sk

    h1 = [aggregate(table1, k) for k in range(N_CORES)]
    allh1 = np.concatenate(h1, axis=0)
    mean = allh1.sum(axis=0) / n_real
    var = (allh1 * allh1).sum(axis=0) / n_real - mean * mean
    inv = 1.0 / np.sqrt(var + BN_EPS)
    g2 = gamma * inv
    b2 = beta - mean * g2

    table2 = np.zeros((N_CORES * npc, D_HID), dtype=np.float32)
    for k in range(N_CORES):
        sk = per_core[k]["s_arr"].T.reshape(-1, 1)
        h2 = np.maximum(h1[k] * g2[None, :] + b2[None, :], 0.0)
        table2[k * npc:(k + 1) * npc] = h2 * sk

    outs = []
    for k in range(N_CORES):
        u = aggregate(table2, k)
        outs.append((u @ wcat + bcat.reshape(1, -1)).astype(np.float32))
    return outs


# ----------------------------------------------------------------------------
# Device program
# ----------------------------------------------------------------------------


def _build_program(geom):
    from concourse import bacc, bass, mybir, tile
    from concourse.masks import make_identity

    (npc, blocks, D_lo, calls_lo, ct_lo, D_hi, calls_hi, ct_hi, n_real,
     call_cols, lo_rows) = geom
    D_lo, D_hi = list(D_lo), list(D_hi)
    f32 = mybir.dt.float32
    bf16 = mybir.dt.bfloat16
    i16 = mybir.dt.int16
    n_tab = N_CORES * npc

    nc = bacc.Bacc("TRN2", target_bir_lowering=False, debug=False,
                   num_devices=N_CORES, num_swdge_queues=NUM_Q)

    t_xsT = nc.dram_tensor("xsT", [P, n_tab], bf16, kind="ExternalInput")
    t_xso = nc.dram_tensor("xsTo", [P, npc], bf16, kind="ExternalInput")
    t_W1 = nc.dram_tensor("W1", [P, D_HID], bf16, kind="ExternalInput")
    t_Wcat = nc.dram_tensor("Wcat", [D_HID, P], bf16, kind="ExternalInput")
    t_bcat = nc.dram_tensor("bcat", [P, 1], f32, kind="ExternalInput")
    t_sarr = nc.dram_tensor("s_arr", [P, blocks], f32, kind="ExternalInput")
    t_wlo = nc.dram_tensor("widx_lo", [P, 8 * ct_lo], i16, kind="ExternalInput")
    t_whi = nc.dram_tensor("widx_hi", [P, 8 * ct_hi], i16, kind="ExternalInput")
    t_wcb = nc.dram_tensor("widx_cb", [P, npc // 16], i16, kind="ExternalInput")
    t_gamma = nc.dram_tensor("gamma", [D_HID, 1], f32, kind="ExternalInput")
    t_beta = nc.dram_tensor("beta", [D_HID, 1], f32, kind="ExternalInput")
    t_out = nc.dram_tensor("out_cat", [P, npc], bf16, kind="ExternalOutput")

    lo_prows = LO_CORES * P
    hi_prows = (N_CORES - LO_CORES) * P
    tab1lo = nc.dram_tensor("tab1lo", [lo_prows, npc], bf16)
    tab1hi = nc.dram_tensor("tab1hi", [hi_prows, npc], bf16)
    ag2_in = nc.dram_tensor("ag2_in", [P, npc], bf16)
    tab2 = nc.dram_tensor("tab2", [N_CORES * P, npc], bf16,
                          addr_space="Shared")
    acc1 = nc.dram_tensor("acc_hi1", [P, npc], bf16)
    acc2 = nc.dram_tensor("acc_hi2", [P, npc], bf16)
    st_in = nc.dram_tensor("st_in", [2 * D_HID], f32)
    st_out = nc.dram_tensor("st_out", [N_CORES * 2 * D_HID], f32,
                            addr_space="Shared")

    groups = [list(range(N_CORES))]
    inv_n = 1.0 / float(n_real)
    xc = XCHUNK_BLOCKS
    n_xchunks = (N_CORES * blocks + xc - 1) // xc

    with tile.TileContext(nc) as tc:
        with (
            tc.tile_pool(name="persist", bufs=1) as persist,
            tc.tile_pool(name="xstream", bufs=3) as xstream,
            tc.tile_pool(name="stage", bufs=3) as stagep,
            tc.tile_pool(name="stream", bufs=4) as stream,
            tc.tile_pool(name="gath", bufs=3) as gath,
            tc.tile_pool(name="gathcb", bufs=1) as gathcb,
            tc.tile_pool(name="small", bufs=1) as small,
            tc.tile_pool(name="ps", bufs=2, space="PSUM") as psp,
            tc.tile_pool(name="ps_stg", bufs=2, space="PSUM") as psstg,
            tc.tile_pool(name="ps_blk", bufs=3, space="PSUM") as psblk,
            tc.tile_pool(name="ps_acc", bufs=1, space="PSUM") as psacc,
        ):
            H = persist.tile([P, npc], f32, tag="H")
            ownslab = persist.tile([P, npc], bf16, tag="ownslab")
            ag2sb = persist.tile([P, npc], bf16, tag="ag2sb")
            accsb = persist.tile([P, npc], bf16, tag="accsb")
            outsb = persist.tile([P, npc], bf16, tag="outsb")
            wlo = persist.tile([P, 8 * ct_lo], i16, tag="wlo")
            whi = persist.tile([P, 8 * ct_hi], i16, tag="whi")
            wcb = persist.tile([P, npc // 16], i16, tag="wcb")
            w1 = small.tile([P, D_HID], bf16, tag="w1")
            wcat = small.tile([D_HID, P], bf16, tag="wcat")
            sarr = small.tile([P, blocks], f32, tag="sarr")
            gcol = small.tile([D_HID, 1], f32, tag="gcol")
            bcol = small.tile([D_HID, 1], f32, tag="bcol")
            bccol = small.tile([P, 1], f32, tag="bccol")
            ident = small.tile([P, P], bf16, tag="ident")
            identf = small.tile([P, P], f32, tag="identf")
            ones_col = small.tile([P, 1], f32, tag="ones_col")
            ones_row = small.tile([1, P], f32, tag="ones_row")
            grep = small.tile([P, P], f32, tag="grep")
            brep = small.tile([P, P], f32, tag="brep")
            bcrep = small.tile([P, P], f32, tag="bcrep")

            nc.sync.dma_start(out=wlo[:], in_=t_wlo[:])
            nc.sync.dma_start(out=whi[:], in_=t_whi[:])
            nc.sync.dma_start(out=wcb[:], in_=t_wcb[:])
            nc.sync.dma_start(out=w1[:], in_=t_W1[:])
            nc.sync.dma_start(out=wcat[:], in_=t_Wcat[:])
            nc.sync.dma_start(out=sarr[:], in_=t_sarr[:])
            nc.sync.dma_start(out=gcol[:], in_=t_gamma[:])
            nc.sync.dma_start(out=bcol[:], in_=t_beta[:])
            nc.sync.dma_start(out=bccol[:], in_=t_bcat[:])
            make_identity(nc, ident[:])
            make_identity(nc, identf[:])
            nc.vector.memset(ones_col[:], 1.0)
            nc.vector.memset(ones_row[:], 1.0)

            def outer_bcast(col_ap, dst_tile):
                pst = psp.tile([P, P], f32, space="PSUM", tag="ps_big")
                nc.tensor.transpose(out=pst[0:1, :], in_=col_ap,
                                    identity=identf[:])
                row = stream.tile([1, P], f32, tag="rowbuf")
                nc.vector.tensor_copy(out=row[:], in_=pst[0:1, :])
                psb = psp.tile([P, P], f32, space="PSUM", tag="ps_big")
                nc.tensor.matmul(out=psb[:], lhsT=ones_row[:], rhs=row[:],
                                 start=True, stop=True)
                nc.vector.tensor_copy(out=dst_tile[:], in_=psb[:])

            outer_bcast(bccol[:], bcrep)

            # --- stage 1 (replicated): tab1 = ((s*x) @ W1) for ALL nodes ----
            # chunks never cross a core-slab boundary so each write is one
            # plain 2-D contiguous-inner slice of tab1lo/tab1hi.
            for k in list(range(LO_CORES, N_CORES)) + list(range(LO_CORES)):
                for c0 in range(0, blocks, xc):
                    c1 = min(c0 + xc, blocks)
                    nb = c1 - c0
                    b0 = k * blocks + c0
                    xt = xstream.tile([P, xc * P], bf16, tag="xt")
                    nc.sync.dma_start(out=xt[:, : nb * P],
                                      in_=t_xsT[:, b0 * P : (b0 + nb) * P])
                    stg = stagep.tile([P, xc * P], bf16, tag="stg")
                    for g0 in range(0, nb, 4):
                        g1 = min(g0 + 4, nb)
                        ps = psstg.tile([P, 4 * P], f32, space="PSUM",
                                        tag="pstg")
                        for i in range(g1 - g0):
                            nc.tensor.matmul(
                                out=ps[:, i * P : (i + 1) * P],
                                lhsT=xt[:, (g0 + i) * P : (g0 + i + 1) * P],
                                rhs=w1[:], start=True, stop=True)
                        nc.vector.tensor_copy(out=stg[:, g0 * P : g1 * P],
                                              in_=ps[:, : (g1 - g0) * P])
                    if k < LO_CORES:
                        dst = tab1lo[k * P : (k + 1) * P,
                                     c0 * P : c1 * P]
                    else:
                        dst = tab1hi[(k - LO_CORES) * P
                                     : (k - LO_CORES + 1) * P,
                                     c0 * P : c1 * P]
                    nc.sync.dma_start(out=dst, in_=stg[:, : nb * P])

            # own slab: recompute from the core's own x slice (tiny)
            for c in range(0, blocks, xc):
                c1 = min(c + xc, blocks)
                nb = c1 - c
                xt = xstream.tile([P, xc * P], bf16, tag="xt")
                nc.sync.dma_start(out=xt[:, : nb * P],
                                  in_=t_xso[:, c * P : c1 * P])
                for g0 in range(0, nb, 4):
                    g1 = min(g0 + 4, nb)
                    ps = psstg.tile([P, 4 * P], f32, space="PSUM", tag="pstg")
                    for i in range(g1 - g0):
                        nc.tensor.matmul(
                            out=ps[:, i * P : (i + 1) * P],
                            lhsT=xt[:, (g0 + i) * P : (g0 + i + 1) * P],
                            rhs=w1[:], start=True, stop=True)
                    nc.vector.tensor_copy(
                        out=ownslab[:, (c + g0) * P : (c + g1) * P],
                        in_=ps[:, : (g1 - g0) * P])

            ps_stats = psacc.tile([D_HID, 2], f32, space="PSUM",
                                  tag="ps_stats")

            qn = [0]

            def next_q():
                q = qn[0]
                qn[0] = (qn[0] + 1) % NUM_Q
                return q

            def gather(out_ap, table_ap, widx_ap, n_idx):
                nc.gpsimd.dma_gather(
                    out_ap.rearrange("p (c f) -> p c f", f=P),
                    table_ap.rearrange("q (r f) -> (q r) f", f=P),
                    widx_ap,
                    num_idxs=n_idx,
                    num_idxs_reg=n_idx,
                    elem_size=P,
                    queue_num=next_q(),
                )

            GB = 4  # blocks per PSUM supergroup

            def aggregate(tlo, thi, acc_t, selfslab, layer):
                # ---- HI pass: per-block PSUM sums (4 blocks/bank) -> accsb
                grp = {}

                def blk_view(b, tag, pre):
                    g, r = b // GB, b % GB
                    if g not in grp:
                        grp[g] = psblk.tile([P, GB * P], f32, space="PSUM",
                                            tag="ps_blk",
                                            name=f"{pre}_{layer}_{g}")
                    return grp[g][:, r * P : (r + 1) * P]

                g_open = set()

                def g_last(b):
                    return min((b // GB) * GB + GB - 1, blocks - 1)

                def flags(b, is_first_mm, is_last_mm):
                    g = b // GB
                    start = is_first_mm and g not in g_open
                    if start:
                        g_open.add(g)
                    stop = is_last_mm and b == g_last(b)
                    return start, stop

                for c0, cols, pieces in calls_hi:
                    gt = gath.tile([P, call_cols * P], bf16, tag="gt")
                    gather(gt[:, : cols * P], thi,
                           whi[:, 8 * c0 : 8 * (c0 + cols)], cols * P)
                    for b, o, d, first, last in pieces:
                        bv = blk_view(b, "ps_blk", "psg_hi")
                        for i in range(d):
                            st_, sp_ = flags(b, first and i == 0,
                                             last and i == d - 1)
                            nc.tensor.matmul(
                                out=bv, lhsT=ident[:],
                                rhs=gt[:, (o + i) * P : (o + i + 1) * P],
                                start=st_, stop=sp_)
                        if last and b == g_last(b):
                            g = b // GB
                            w = (b % GB) + 1
                            nc.vector.tensor_copy(
                                out=accsb[:, g * GB * P : g * GB * P + w * P],
                                in_=grp.pop(g)[:, : w * P])
                nc.sync.dma_start(out=acc_t[:, :], in_=accsb[:])

                # ---- combine gather: cbt[:, j] = acc_t[himap[j]]
                cbt = gathcb.tile([P, npc], bf16, tag="gtcb",
                                  name=f"gtcb_{layer}")
                for cb0 in range(0, blocks, call_cols):
                    cb1 = min(cb0 + call_cols, blocks)
                    gather(cbt[:, cb0 * P : cb1 * P], acc_t,
                           wcb[:, 8 * cb0 : 8 * cb1], (cb1 - cb0) * P)

                # ---- LO pass + self + cb into per-block PSUM, then scale
                grp.clear()
                g_open.clear()

                def start_block(b):
                    bv = blk_view(b, "ps_blk", "psg_lo")
                    st_, _ = flags(b, True, False)
                    nc.tensor.matmul(out=bv, lhsT=ident[:],
                                     rhs=selfslab[:, b * P : (b + 1) * P],
                                     start=st_, stop=False)
                    nc.tensor.matmul(out=bv, lhsT=ident[:],
                                     rhs=cbt[:, b * P : (b + 1) * P],
                                     start=False, stop=False)
                    return bv

                for c0, cols, pieces in calls_lo:
                    gt = gath.tile([P, call_cols * P], bf16, tag="gt")
                    gather(gt[:, : cols * P], tlo,
                           wlo[:, 8 * c0 : 8 * (c0 + cols)], cols * P)
                    for b, o, d, first, last in pieces:
                        if first:
                            start_block(b)
                        bv = blk_view(b, "ps_blk", "psg_lo")
                        for i in range(d):
                            _, sp_ = flags(b, False, last and i == d - 1)
                            nc.tensor.matmul(
                                out=bv, lhsT=ident[:],
                                rhs=gt[:, (o + i) * P : (o + i + 1) * P],
                                start=False, stop=sp_)
                        if last and b == g_last(b):
                            g = b // GB
                            gt_ps = grp.pop(g)
                            for r in range(b % GB + 1):
                                bb = g * GB + r
                                sl = slice(bb * P, (bb + 1) * P)
                                nc.vector.tensor_scalar_mul(
                                    H[:, sl], gt_ps[:, r * P : (r + 1) * P],
                                    sarr[:, bb : bb + 1])
                                if layer == 1:
                                    sq = stream.tile([P, D_HID], f32,
                                                     tag="sq")
                                    nc.scalar.square(out=sq[:], in_=H[:, sl])
                                    nc.tensor.matmul(
                                        out=ps_stats[:, 0:1], lhsT=H[:, sl],
                                        rhs=ones_col[:],
                                        start=(bb == 0), stop=False)
                                    nc.tensor.matmul(
                                        out=ps_stats[:, 1:2], lhsT=sq[:],
                                        rhs=ones_col[:],
                                        start=False,
                                        stop=(bb == blocks - 1))

            aggregate(tab1lo, tab1hi, acc1, ownslab, layer=1)

            # --- BN ---------------------------------------------------------
            st = small.tile([D_HID, 2], f32, tag="st")
            nc.vector.tensor_copy(out=st[:], in_=ps_stats[:])
            nc.sync.dma_start(out=st_in[:], in_=st[:])
            nc.gpsimd.collective_compute(
                "AllGather", mybir.AluOpType.bypass, replica_groups=groups,
                ins=[st_in[:]], outs=[st_out[:]],
            )
            st8 = small.tile([D_HID, 2 * N_CORES], f32, tag="st8")
            nc.sync.dma_start(
                out=st8.rearrange("p (c t) -> p c t", t=2),
                in_=st_out[:].rearrange("(c p t) -> p c t", p=D_HID, t=2),
            )
            st2 = small.tile([D_HID, 2], f32, tag="st2")
            nc.vector.reduce_sum(
                out=st2[:],
                in_=st8.rearrange("p (c t) -> p t c", t=2),
                axis=mybir.AxisListType.X)

            eps_col = small.tile([D_HID, 1], f32, tag="eps_col")
            nc.vector.memset(eps_col[:], BN_EPS)
            mean = small.tile([D_HID, 1], f32, tag="mean")
            msq = small.tile([D_HID, 1], f32, tag="msq")
            var = small.tile([D_HID, 1], f32, tag="var")
            std = small.tile([D_HID, 1], f32, tag="std")
            istd = small.tile([D_HID, 1], f32, tag="istd")
            gp = small.tile([D_HID, 1], f32, tag="gp")
            bp_ = small.tile([D_HID, 1], f32, tag="bp")
            nc.vector.tensor_scalar_mul(mean[:], st2[:, 0:1], inv_n)
            nc.vector.tensor_scalar_mul(msq[:], st2[:, 1:2], inv_n)
            nc.scalar.square(out=var[:], in_=mean[:])
            nc.vector.tensor_tensor(out=var[:], in0=msq[:], in1=var[:],
                                    op=mybir.AluOpType.subtract)
            nc.scalar.activation(out=std[:], in_=var[:],
                                 func=mybir.ActivationFunctionType.Sqrt,
                                 bias=eps_col[:])
            nc.vector.reciprocal(out=istd[:], in_=std[:])
            nc.vector.tensor_tensor(out=gp[:], in0=gcol[:], in1=istd[:],
                                    op=mybir.AluOpType.mult)
            nc.vector.tensor_tensor(out=bp_[:], in0=mean[:], in1=gp[:],
                                    op=mybir.AluOpType.mult)
            nc.vector.tensor_tensor(out=bp_[:], in0=bcol[:], in1=bp_[:],
                                    op=mybir.AluOpType.subtract)
            outer_bcast(gp[:], grep)
            outer_bcast(bp_[:], brep)

            grep4 = small.tile([P, 4 * P], f32, tag="grep4")
            brep4 = small.tile([P, 4 * P], f32, tag="brep4")
            for r in range(4):
                nc.vector.tensor_copy(out=grep4[:, r * P : (r + 1) * P],
                                      in_=grep[:])
                nc.vector.tensor_copy(out=brep4[:, r * P : (r + 1) * P],
                                      in_=brep[:])
            for b0 in range(0, blocks, 4):
                b1 = min(b0 + 4, blocks)
                w = (b1 - b0) * P
                sl4 = slice(b0 * P, b1 * P)
                t1 = stream.tile([P, 4 * P], f32, tag="bn1")
                nc.vector.tensor_tensor(out=t1[:, :w], in0=H[:, sl4],
                                        in1=grep4[:, :w],
                                        op=mybir.AluOpType.mult)
                nc.vector.tensor_tensor(out=t1[:, :w], in0=t1[:, :w],
                                        in1=brep4[:, :w],
                                        op=mybir.AluOpType.add)
                nc.scalar.activation(out=t1[:, :w], in_=t1[:, :w],
                                     func=mybir.ActivationFunctionType.Relu)
                for b in range(b0, b1):
                    r = b - b0
                    nc.vector.tensor_scalar_mul(
                        ag2sb[:, b * P : (b + 1) * P],
                        t1[:, r * P : (r + 1) * P], sarr[:, b : b + 1])
            nc.sync.dma_start(out=ag2_in[:, :], in_=ag2sb[:])

            nc.gpsimd.collective_compute(
                "AllGather", mybir.AluOpType.bypass, replica_groups=groups,
                ins=[ag2_in[:]], outs=[tab2[:]],
            )

            aggregate(tab2[: LO_CORES * P, :],
                      tab2[LO_CORES * P :, :], acc2, ag2sb,
                      layer=2)

            for b in range(blocks):
                sl = slice(b * P, (b + 1) * P)
                pst = psp.tile([P, P], f32, space="PSUM", tag="ps_big")
                nc.tensor.transpose(out=pst[:], in_=H[:, sl],
                                    identity=identf[:])
                uT = stream.tile([P, P], bf16, tag="uT")
                nc.vector.tensor_copy(out=uT[:], in_=pst[:])
                pso = psp.tile([P, P], f32, space="PSUM", tag="ps_big")
                nc.tensor.matmul(out=pso[:], lhsT=uT[:], rhs=wcat[:],
                                 start=True, stop=True)
                nc.vector.tensor_add(out=outsb[:, sl], in0=pso[:],
                                     in1=bcrep[:])
            nc.sync.dma_start(out=t_out[:, :], in_=outsb[:])

    nc.compile()
    return nc


# ----------------------------------------------------------------------------
# Entry point
# ----------------------------------------------------------------------------

_IN_NAMES = ["xsT", "xsTo", "W1", "Wcat", "bcat", "s_arr", "widx_lo",
             "widx_hi", "widx_cb", "gamma", "beta"]


def _geom(plan, call_cols):
    return (
        plan["npc"],
        plan["blocks"],
        tuple(int(d) for d in plan["D_lo"]),
        tuple(plan["calls_lo"]),
        plan["ct_lo"],
        tuple(int(d) for d in plan["D_hi"]),
        tuple(plan["calls_hi"]),
        plan["ct_hi"],
        int(plan["node_of"].max()) + 1,
        call_cols,
        plan["lo_rows"],
    )


def _run_hw(nc, per_core, trace=False, trace_cores=None):
    from concourse import bass_utils

    in_maps = [{nm: per_core[k][nm] for nm in _IN_NAMES} for k in range(N_CORES)]
    res = bass_utils.run_bass_kernel_spmd(
        nc, in_maps, core_ids=list(range(N_CORES)), trace=trace,
        trace_cores=trace_cores,
    )
    outs = [res.results[k]["out_cat"] for k in range(N_CORES)]
    return outs, res


def kernel(x, edge_index, W1, b1, gamma, beta, Wmu, bmu, Wls, bls):
    x = np.asarray(x, dtype=np.float32)
    edge_index = np.asarray(edge_index)
    W1 = np.asarray(W1, dtype=np.float32)
    gamma = np.asarray(gamma, dtype=np.float32)
    beta = np.asarray(beta, dtype=np.float32)
    Wmu = np.asarray(Wmu, dtype=np.float32)
    bmu = np.asarray(bmu, dtype=np.float32)
    Wls = np.asarray(Wls, dtype=np.float32)
    bls = np.asarray(bls, dtype=np.float32)

    plan = _plan(edge_index, x.shape[0], N_CORES, call_cols=CALL_COLS)
    per_core = _host_inputs(plan, x, W1, Wmu, Wls, bmu, bls, gamma, beta)

    geom = _geom(plan, CALL_COLS)
    if geom not in _CACHE:
        _CACHE[geom] = _build_program(geom)
    nc = _CACHE[geom]

    outs, _ = _run_hw(nc, per_core, trace=False)
    mu, ls = _postprocess(plan, outs)
    return mu, ls


# revision 23
# speedup vs baseline: 1.1371x; 1.1371x over previous
"""GCN encoder (2x GCNConv + BatchNorm/ReLU) on 8 Trainium2 NeuronCores.

Math: with s = 1/sqrt(deg+1) (deg = in-degree by dst), the GCN edge norm
factorizes: norm_e = s[src]*s[dst], so for any node features H,
    A(H) := segsum(norm_e * H[src], dst) + H * s^2
          = s * ( segsum( (s*H)[src], dst) + (s*H) )
and GCNConv(H, W, b) = A(H)@W + b = A(H@W) + b, so the whole net needs only
TWO sparse aggregations (layer1 on (s*x)@W1, layer2 on s*post-BN hidden),
and mu / log_std share the second one.

v2 design (vs the AllGather-everything baseline):
  * Stage 1 is REPLICATED: every core streams the full (s-prescaled, bf16)
    x^T from its local DRAM and computes the whole 50176-row message table
    with the otherwise-idle PE -> no AllGather #1 (saves a ~250us collective).
  * Message tables are bf16 (halves collective + HBM gather traffic).
  * Segment sums run on the PE: the gather layout puts slot-aligned edges on
    partitions, so each 128-column slot is accumulated into a per-block PSUM
    region with an identity-matmul (PSUM fp32 accumulate; 4 blocks share one
    2KB bank as a single accumulation group) -- DVE only does one scale per
    block.  Padding slots point at guaranteed-zero table rows.
  * All tables are 2-D partition-major ([core*128+p, block*128+f]), so every
    table/accumulator DMA is a plain contiguous slice at full line rate (no
    256B-descriptor penalty), and lo/hi halves are contiguous row ranges.
  * Gather call size stays at 7 cols * 128 = 896 indices (the hw SWDGE
    descriptor ring holds 128 in-flight entries per engine and a call needs
    num_idxs*2/16+1, so num_idxs <= ~1000; larger calls hang the device).
    The calls are off the critical path -- PE streaming and the AllGather
    bound the sections -- so the small call size costs nothing.
  * Self-loop term is added from an SBUF-resident copy of the core's own
    table slab (recomputed from a small per-core x slice) via one more
    identity-matmul per block.
  * BN batch stats go through a tiny AllGather (cheaper than AllReduce) and
    are combined locally.  Only layer 2 still needs a real table AllGather.

Host preprocessing (free): integer edge plan, permutations, index tensors,
plus input marshaling: x is row-scaled by s, permuted to table order, and
cast to bf16 ((diag(s) x) @ W1 == s * (x @ W1)).  All O(N*D^2) and per-edge
float math stays on device.
"""

import numpy as np

N_NODES = 50000
N_EDGES = 800000
D_IN = 128
D_HID = 128
D_LAT = 64
BN_EPS = 1e-5
N_CORES = 8
P = 128
LO_CORES = 5  # cores 0..4 form the "lo" table half; 5*6272=31360 < 32768
              # (dma_gather int16 indices address at most 32768 rows per call)

CALL_COLS = 7      # gather call size: 7 cols * 128 = 896 idxs (hw ring cap)
NUM_Q = 4          # SWDGE queues

_CACHE = {}


# ----------------------------------------------------------------------------
# Host-side preprocessing
# ----------------------------------------------------------------------------


def _wrap_idx(lin):
    """dma_gather idx layout: position i -> [i%16, i//16], replicated to 128
    partitions. lin: [n] int array (n % 16 == 0) -> [128, n//16] int16."""
    n = lin.shape[0]
    w = lin.reshape(n // 16, 16).T.astype(np.int16)  # [16, n//16]
    return np.tile(w, (8, 1))


def _pack_calls(D, call_cols):
    """Slice the global column space into calls of <= call_cols columns.

    A call may cover partial blocks; each call carries its piece list
    [(block, col_off_in_call, width, first, last)].
    """
    C0 = np.concatenate([[0], np.cumsum(D)]).astype(np.int64)
    ct = int(C0[-1])
    calls = []
    for c0 in range(0, ct, call_cols):
        c1 = min(c0 + call_cols, ct)
        pieces = []
        for b in range(len(D)):
            lo = max(c0, int(C0[b]))
            hi = min(c1, int(C0[b + 1]))
            if lo < hi:
                pieces.append(
                    (b, lo - c0, hi - lo, lo == int(C0[b]), hi == int(C0[b + 1]))
                )
        calls.append((c0, c1 - c0, tuple(pieces)))
    return C0, calls


def _build_pass(tcoord_src, tkey_dst, n_cores, npc, blocks, call_cols,
                pad_idx, idx_base):
    """Build one gather pass layout.

    tcoord_src: per-edge source table coord (already offset for hi pass)
    tkey_dst:   per-edge dst node key in THIS pass's permutation
    Returns D [blocks], C0, calls, idx arrays [n_cores, 128, c_total] int32.
    """
    deg = np.bincount(tkey_dst, minlength=n_cores * npc)
    d3 = deg.reshape(n_cores, blocks, P)
    D = d3.max(axis=(0, 2)).astype(np.int64)
    D = np.maximum(D, 1)
    C0, calls = _pack_calls(D, call_cols)
    c_total = int(C0[-1])

    idx = np.full((n_cores, P, c_total), pad_idx - idx_base, dtype=np.int32)
    eorder = np.argsort(tkey_dst, kind="stable")
    k_s = tkey_dst[eorder]
    src_s = (tcoord_src[eorder] - idx_base).astype(np.int32)
    grp = np.searchsorted(k_s, k_s)
    slot = np.arange(k_s.size) - grp
    core_e = k_s // npc
    local_e = k_s % npc
    b_e = local_e // P
    p_e = local_e % P
    col_e = C0[b_e] + slot
    assert (slot < D[b_e]).all()
    idx[core_e, p_e, col_e] = src_s
    return D, C0, calls, idx, c_total


def _idx_to_wrapped(idx):
    """[n_cores, 128, c_total] int32 -> wrapped int16 [n_cores, 128, 8*c_total].

    Global linear position order is column-major (i = c*128 + p); contiguous
    position chunks map to contiguous wrapped columns, so any call covering
    cols [c0, c1) reads the wrapped slice [:, 8*c0 : 8*c1]."""
    n_cores, _, c_total = idx.shape
    out = np.empty((n_cores, 128, 8 * c_total), dtype=np.int16)
    for k in range(n_cores):
        lin = idx[k].T.reshape(-1)
        out[k] = _wrap_idx(lin)
    return out


def _plan(edge_index, n_nodes, n_cores, call_cols):
    src = np.asarray(edge_index[0], dtype=np.int64)
    dst = np.asarray(edge_index[1], dtype=np.int64)

    deg_in = np.bincount(dst, minlength=n_nodes).astype(np.int64)
    s = (1.0 / np.sqrt((deg_in + 1).astype(np.float64))).astype(np.float32)

    n_local = (n_nodes + n_cores - 1) // n_cores
    blocks = (n_local + 1 + P - 1) // P
    npc = blocks * P
    lo_rows = LO_CORES * npc
    assert lo_rows < 32768 and (n_cores * npc - lo_rows) < 32768

    # ---- core assignment: deal by total-degree rank (balances edge load and
    # aligns block-degree profiles across cores).
    order = np.argsort(-deg_in, kind="stable")
    rank_of = np.empty(n_nodes, dtype=np.int64)
    rank_of[order] = np.arange(n_nodes)
    core_of = rank_of % n_cores

    src_is_lo = core_of[src] < LO_CORES
    dlo = np.bincount(dst[src_is_lo], minlength=n_nodes)
    dhi = np.bincount(dst[~src_is_lo], minlength=n_nodes)

    # main layout: per-core locals sorted by lo-degree (tight LO padding)
    local_of = np.empty(n_nodes, dtype=np.int64)
    node2hi = np.empty(n_nodes, dtype=np.int64)
    for k in range(n_cores):
        nodes_k = np.nonzero(core_of == k)[0]
        o = nodes_k[np.argsort(-dlo[nodes_k], kind="stable")]
        local_of[o] = np.arange(o.size)
        o2 = nodes_k[np.argsort(-dhi[nodes_k], kind="stable")]
        node2hi[o2] = k * npc + np.arange(o2.size)
    node2table = core_of * npc + local_of


    # ---- gather-source row numbering: tables are [(core,part), (block,feat)]
    # 2-D tensors, so node (core k, local j=b*128+p) lives at flat row
    # (k*128+p)*blocks + b of its half (hi half: k-LO_CORES).
    def kpb_row(core, local, core0):
        return ((core - core0) * P + local % P) * blocks + local // P

    node2row = np.where(
        core_of < LO_CORES,
        kpb_row(core_of, local_of, 0),
        kpb_row(core_of, local_of, LO_CORES),
    )
    pad_lo = kpb_row(0, npc - 1, 0)
    pad_hi = kpb_row(N_CORES - 1, npc - 1, LO_CORES)

    # ---- LO pass on the main permutation
    D_lo, C0_lo, calls_lo, idx_lo, ct_lo = _build_pass(
        node2row[src[src_is_lo]], node2table[dst[src_is_lo]], n_cores, npc,
        blocks, call_cols, pad_lo, 0,
    )

    # ---- HI pass on the hi permutation
    D_hi, C0_hi, calls_hi, idx_hi, ct_hi = _build_pass(
        node2row[src[~src_is_lo]], node2hi[dst[~src_is_lo]], n_cores, npc,
        blocks, call_cols, pad_hi, 0,
    )

    # ---- combine map: main-layout local j gets acc_hi[himap[j]] added
    himap = np.full((n_cores, npc), npc - 1, dtype=np.int64)  # pad -> pad row
    for k in range(n_cores):
        nodes_k = np.nonzero(core_of == k)[0]
        himap[k, local_of[nodes_k]] = node2hi[nodes_k] % npc

    widx_lo = _idx_to_wrapped(idx_lo)
    widx_hi = _idx_to_wrapped(idx_hi)
    himap_row = (himap % P) * blocks + himap // P
    widx_cb = np.stack([_wrap_idx(himap_row[k]) for k in range(n_cores)])

    # per-core node lists and s in the MAIN layout
    node_of = np.full((n_cores, npc), -1, dtype=np.int64)
    s_arr = np.zeros((n_cores, P, blocks), dtype=np.float32)
    for k in range(n_cores):
        nodes_k = np.nonzero(core_of == k)[0]
        loc = local_of[nodes_k]
        node_of[k, loc] = nodes_k
        s_arr[k, loc % P, loc // P] = s[nodes_k]

    return dict(
        s=s,
        node2table=node2table,
        node_of=node_of,
        npc=npc,
        blocks=blocks,
        n_local=n_local,
        lo_rows=lo_rows,
        s_arr=s_arr,
        D_lo=D_lo, C0_lo=C0_lo, calls_lo=calls_lo, idx_lo=idx_lo, ct_lo=ct_lo,
        D_hi=D_hi, C0_hi=C0_hi, calls_hi=calls_hi, idx_hi=idx_hi, ct_hi=ct_hi,
        himap=himap,
        widx_lo=widx_lo, widx_hi=widx_hi, widx_cb=widx_cb,
    )


def _to_bf16(a):
    import ml_dtypes

    return a.astype(ml_dtypes.bfloat16)


def _host_inputs(plan, x, W1, Wmu, Wls, bmu, bls, gamma, beta):
    npc = plan["npc"]
    node_of = plan["node_of"]
    s = plan["s"]
    wcat = np.concatenate([Wmu, Wls], axis=1).astype(np.float32)
    bcat = np.concatenate([bmu, bls]).astype(np.float32).reshape(D_HID, 1)

    # s-prescaled x in the partition-major table layout shared with tab2:
    # row (core*128 + p), col (block*128 + f) holds node (core, b*128+p).
    blocks = npc // P
    xs_tab = np.zeros((N_CORES * npc, x.shape[1]), dtype=np.float32)
    for k in range(N_CORES):
        nodes = node_of[k]
        valid = nodes >= 0
        xs_tab[k * npc + np.nonzero(valid)[0]] = (
            x[nodes[valid]] * s[nodes[valid]][:, None]
        )
    xs_kpb = (
        xs_tab.reshape(N_CORES, blocks, P, D_IN)
        .transpose(0, 2, 1, 3)
        .reshape(N_CORES * P, blocks * D_IN)
    )
    xs_kpb = _to_bf16(xs_kpb)
    xs_lo = np.ascontiguousarray(xs_kpb[: LO_CORES * P])
    xs_hi = np.ascontiguousarray(xs_kpb[LO_CORES * P :])

    per_core = []
    for k in range(N_CORES):
        per_core.append(
            {
                "xslo": xs_lo,
                "xshi": xs_hi,
                "xso": np.ascontiguousarray(xs_kpb[k * P : (k + 1) * P]),
                "W1": np.ascontiguousarray(_to_bf16(W1)),
                "Wcat": np.ascontiguousarray(_to_bf16(wcat)),
                "bcat": bcat,
                "s_arr": np.ascontiguousarray(plan["s_arr"][k]),
                "widx_lo": np.ascontiguousarray(plan["widx_lo"][k]),
                "widx_hi": np.ascontiguousarray(plan["widx_hi"][k]),
                "widx_cb": np.ascontiguousarray(plan["widx_cb"][k]),
                "gamma": gamma.astype(np.float32).reshape(D_HID, 1),
                "beta": beta.astype(np.float32).reshape(D_HID, 1),
            }
        )
    return per_core


def _postprocess(plan, outs):
    n_nodes = int(plan["node_of"].max()) + 1
    mu = np.zeros((n_nodes, D_LAT), dtype=np.float32)
    ls = np.zeros((n_nodes, D_LAT), dtype=np.float32)
    node_of = plan["node_of"]
    npc = node_of.shape[1]
    blocks = npc // 128
    for k in range(N_CORES):
        nodes = node_of[k]
        valid = nodes >= 0
        o = np.asarray(outs[k]).astype(np.float32).reshape(128, blocks, 128)
        o = o.transpose(1, 0, 2).reshape(npc, 128)  # node-major (b*128+p)
        mu[nodes[valid]] = o[valid.nonzero()[0], :D_LAT]
        ls[nodes[valid]] = o[valid.nonzero()[0], D_LAT:]
    return mu, ls


# ----------------------------------------------------------------------------
# Numpy emulation of the device program (fp32; bf16 rounding not modeled)
# ----------------------------------------------------------------------------


def _numpy_model(plan, per_core, x, W1, wcat, bcat, gamma, beta, n_real):
    npc, blocks = plan["npc"], plan["blocks"]
    lo_rows = plan["lo_rows"]
    node_of, s = plan["node_of"], plan["s"]

    table1 = np.zeros((N_CORES * npc, D_HID), dtype=np.float32)
    for k in range(N_CORES):
        nodes = node_of[k]
        valid = nodes >= 0
        table1[k * npc + np.nonzero(valid)[0]] = (
            (x[nodes[valid]] * s[nodes[valid]][:, None]) @ W1
        )

    def aggregate(table, k):
        out = np.zeros((npc, table.shape[1]), dtype=np.float32)
        mlo = table[:lo_rows][plan["idx_lo"][k]]  # [P, ct, F]
        C0 = plan["C0_lo"]
        for b in range(blocks):
            out[b * P:(b + 1) * P] += mlo[:, C0[b]:C0[b + 1], :].sum(axis=1)
        mhi = table[lo_rows:][plan["idx_hi"][k]]
        acc_hi = np.zeros((npc, table.shape[1]), dtype=np.float32)
        C0 = plan["C0_hi"]
        for b in range(blocks):
            acc_hi[b * P:(b + 1) * P] = mhi[:, C0[b]:C0[b + 1], :].sum(axis=1)
        out += acc_hi[plan["himap"][k]]
        own0 = k * npc
        sk = per_core[k]["s_arr"].T.reshape(-1, 1)
        return (out + table[own0:own0 + npc]) * sk

    h1 = [aggregate(table1, k) for k in range(N_CORES)]
    allh1 = np.concatenate(h1, axis=0)
    mean = allh1.sum(axis=0) / n_real
    var = (allh1 * allh1).sum(axis=0) / n_real - mean * mean
    inv = 1.0 / np.sqrt(var + BN_EPS)
    g2 = gamma * inv
    b2 = beta - mean * g2

    table2 = np.zeros((N_CORES * npc, D_HID), dtype=np.float32)
    for k in range(N_CORES):
        sk = per_core[k]["s_arr"].T.reshape(-1, 1)
        h2 = np.maximum(h1[k] * g2[None, :] + b2[None, :], 0.0)
        table2[k * npc:(k + 1) * npc] = h2 * sk

    outs = []
    for k in range(N_CORES):
        u = aggregate(table2, k)
        outs.append((u @ wcat + bcat.reshape(1, -1)).astype(np.float32))
    return outs


# ----------------------------------------------------------------------------
# Device program
# ----------------------------------------------------------------------------


def _build_program(geom):
    from concourse import bacc, bass, mybir, tile
    from concourse.masks import make_identity

    (npc, blocks, D_lo, calls_lo, ct_lo, D_hi, calls_hi, ct_hi, n_real,
     call_cols, lo_rows) = geom
    D_lo, D_hi = list(D_lo), list(D_hi)
    f32 = mybir.dt.float32
    bf16 = mybir.dt.bfloat16
    i16 = mybir.dt.int16
    n_tab = N_CORES * npc

    nc = bacc.Bacc("TRN2", target_bir_lowering=False, debug=False,
                   num_devices=N_CORES, num_swdge_queues=NUM_Q)

    t_xslo = nc.dram_tensor("xslo", [LO_CORES * P, npc], bf16,
                            kind="ExternalInput")
    t_xshi = nc.dram_tensor("xshi", [(N_CORES - LO_CORES) * P, npc], bf16,
                            kind="ExternalInput")
    t_xso = nc.dram_tensor("xso", [P, npc], bf16, kind="ExternalInput")
    t_W1 = nc.dram_tensor("W1", [P, D_HID], bf16, kind="ExternalInput")
    t_Wcat = nc.dram_tensor("Wcat", [D_HID, P], bf16, kind="ExternalInput")
    t_bcat = nc.dram_tensor("bcat", [P, 1], f32, kind="ExternalInput")
    t_sarr = nc.dram_tensor("s_arr", [P, blocks], f32, kind="ExternalInput")
    t_wlo = nc.dram_tensor("widx_lo", [P, 8 * ct_lo], i16, kind="ExternalInput")
    t_whi = nc.dram_tensor("widx_hi", [P, 8 * ct_hi], i16, kind="ExternalInput")
    t_wcb = nc.dram_tensor("widx_cb", [P, npc // 16], i16, kind="ExternalInput")
    t_gamma = nc.dram_tensor("gamma", [D_HID, 1], f32, kind="ExternalInput")
    t_beta = nc.dram_tensor("beta", [D_HID, 1], f32, kind="ExternalInput")
    t_out = nc.dram_tensor("out_cat", [P, npc], bf16, kind="ExternalOutput")

    ag2_in = nc.dram_tensor("ag2_in", [P, npc], bf16)
    tab2 = nc.dram_tensor("tab2", [N_CORES * P, npc], bf16,
                          addr_space="Shared")
    acc1 = nc.dram_tensor("acc_hi1", [P, npc], bf16)
    acc2 = nc.dram_tensor("acc_hi2", [P, npc], bf16)
    st_in = nc.dram_tensor("st_in", [2 * D_HID], f32)
    st_out = nc.dram_tensor("st_out", [N_CORES * 2 * D_HID], f32,
                            addr_space="Shared")

    groups = [list(range(N_CORES))]
    inv_n = 1.0 / float(n_real)

    with tile.TileContext(nc) as tc:
        with (
            tc.tile_pool(name="persist", bufs=1) as persist,
            tc.tile_pool(name="stream", bufs=4) as stream,
            tc.tile_pool(name="gath", bufs=3) as gath,
            tc.tile_pool(name="gathcb", bufs=1) as gathcb,
            tc.tile_pool(name="small", bufs=1) as small,
            tc.tile_pool(name="ps", bufs=2, space="PSUM") as psp,
            tc.tile_pool(name="ps_blk", bufs=3, space="PSUM") as psblk,
            tc.tile_pool(name="ps_acc", bufs=1, space="PSUM") as psacc,
        ):
            H = persist.tile([P, npc], bf16, tag="H")
            ownslab = persist.tile([P, npc], bf16, tag="ownslab")
            ag2sb = persist.tile([P, npc], bf16, tag="ag2sb")
            accsb = persist.tile([P, npc], bf16, tag="accsb")
            outsb = persist.tile([P, npc], bf16, tag="outsb")
            wlo = persist.tile([P, 8 * ct_lo], i16, tag="wlo")
            whi = persist.tile([P, 8 * ct_hi], i16, tag="whi")
            wcb = persist.tile([P, npc // 16], i16, tag="wcb")
            w1 = small.tile([P, D_HID], bf16, tag="w1")
            wcat = small.tile([D_HID, P], bf16, tag="wcat")
            sarr = small.tile([P, blocks], f32, tag="sarr")
            gcol = small.tile([D_HID, 1], f32, tag="gcol")
            bcol = small.tile([D_HID, 1], f32, tag="bcol")
            bccol = small.tile([P, 1], f32, tag="bccol")
            ident = small.tile([P, P], bf16, tag="ident")
            identf = small.tile([P, P], f32, tag="identf")
            ones_col = small.tile([P, 1], bf16, tag="ones_col")
            ones_row = small.tile([1, P], f32, tag="ones_row")
            grep = small.tile([P, P], f32, tag="grep")
            brep = small.tile([P, P], f32, tag="brep")
            bcrep = small.tile([P, P], f32, tag="bcrep")

            nc.sync.dma_start(out=wlo[:], in_=t_wlo[:])
            nc.sync.dma_start(out=whi[:], in_=t_whi[:])
            nc.sync.dma_start(out=wcb[:], in_=t_wcb[:])
            nc.sync.dma_start(out=w1[:], in_=t_W1[:])
            nc.sync.dma_start(out=wcat[:], in_=t_Wcat[:])
            nc.sync.dma_start(out=sarr[:], in_=t_sarr[:])
            nc.sync.dma_start(out=gcol[:], in_=t_gamma[:])
            nc.sync.dma_start(out=bcol[:], in_=t_beta[:])
            nc.sync.dma_start(out=bccol[:], in_=t_bcat[:])
            make_identity(nc, ident[:])
            make_identity(nc, identf[:])
            nc.vector.memset(ones_col[:], 1.0)
            nc.vector.memset(ones_row[:], 1.0)

            def outer_bcast(col_ap, dst_tile):
                pst = psp.tile([P, P], f32, space="PSUM", tag="ps_big")
                nc.tensor.transpose(out=pst[0:1, :], in_=col_ap,
                                    identity=identf[:])
                row = stream.tile([1, P], f32, tag="rowbuf")
                nc.vector.tensor_copy(out=row[:], in_=pst[0:1, :])
                psb = psp.tile([P, P], f32, space="PSUM", tag="ps_big")
                nc.tensor.matmul(out=psb[:], lhsT=ones_row[:], rhs=row[:],
                                 start=True, stop=True)
                nc.vector.tensor_copy(out=dst_tile[:], in_=psb[:])

            outer_bcast(bccol[:], bcrep)

            # --- layer 1 gathers raw (s*x): A(x@W1) == A(x)@W1, so W1 is
            # applied per-block AFTER aggregation; the host-marshaled xslo/
            # xshi inputs ARE the gather tables (no stage-1, no table write).
            nc.sync.dma_start(out=ownslab[:], in_=t_xso[:, :])

            ps_stats = psacc.tile([D_HID, 2], f32, space="PSUM",
                                  tag="ps_stats")

            qn = [0]

            def next_q():
                q = qn[0]
                qn[0] = (qn[0] + 1) % NUM_Q
                return q

            def gather(out_ap, table_ap, widx_ap, n_idx):
                nc.gpsimd.dma_gather(
                    out_ap.rearrange("p (c f) -> p c f", f=P),
                    table_ap.rearrange("q (r f) -> (q r) f", f=P),
                    widx_ap,
                    num_idxs=n_idx,
                    num_idxs_reg=n_idx,
                    elem_size=P,
                    queue_num=next_q(),
                )

            GB = 4  # blocks per PSUM supergroup

            def aggregate(tlo, thi, acc_t, selfslab, layer):
                # ---- HI pass: per-block PSUM sums (4 blocks/bank) -> accsb
                grp = {}

                def blk_view(b, tag, pre):
                    g, r = b // GB, b % GB
                    if g not in grp:
                        grp[g] = psblk.tile([P, GB * P], f32, space="PSUM",
                                            tag="ps_blk",
                                            name=f"{pre}_{layer}_{g}")
                    return grp[g][:, r * P : (r + 1) * P]

                g_open = set()

                def g_last(b):
                    return min((b // GB) * GB + GB - 1, blocks - 1)

                def flags(b, is_first_mm, is_last_mm):
                    g = b // GB
                    start = is_first_mm and g not in g_open
                    if start:
                        g_open.add(g)
                    stop = is_last_mm and b == g_last(b)
                    return start, stop

                for c0, cols, pieces in calls_hi:
                    gt = gath.tile([P, call_cols * P], bf16, tag="gt")
                    gather(gt[:, : cols * P], thi,
                           whi[:, 8 * c0 : 8 * (c0 + cols)], cols * P)
                    for b, o, d, first, last in pieces:
                        bv = blk_view(b, "ps_blk", "psg_hi")
                        for i in range(d):
                            st_, sp_ = flags(b, first and i == 0,
                                             last and i == d - 1)
                            nc.tensor.matmul(
                                out=bv, lhsT=ident[:],
                                rhs=gt[:, (o + i) * P : (o + i + 1) * P],
                                start=st_, stop=sp_)
                        if last and b == g_last(b):
                            g = b // GB
                            w = (b % GB) + 1
                            nc.vector.tensor_copy(
                                out=accsb[:, g * GB * P : g * GB * P + w * P],
                                in_=grp.pop(g)[:, : w * P])
                nc.sync.dma_start(out=acc_t[:, :], in_=accsb[:])

                # ---- combine gather: cbt[:, j] = acc_t[himap[j]]
                cbt = gathcb.tile([P, npc], bf16, tag="gtcb",
                                  name=f"gtcb_{layer}")
                for cb0 in range(0, blocks, call_cols):
                    cb1 = min(cb0 + call_cols, blocks)
                    gather(cbt[:, cb0 * P : cb1 * P], acc_t,
                           wcb[:, 8 * cb0 : 8 * cb1], (cb1 - cb0) * P)

                # ---- LO pass + self + cb into per-block PSUM, then scale
                grp.clear()
                g_open.clear()

                def start_block(b):
                    bv = blk_view(b, "ps_blk", "psg_lo")
                    st_, _ = flags(b, True, False)
                    nc.tensor.matmul(out=bv, lhsT=ident[:],
                                     rhs=selfslab[:, b * P : (b + 1) * P],
                                     start=st_, stop=False)
                    nc.tensor.matmul(out=bv, lhsT=ident[:],
                                     rhs=cbt[:, b * P : (b + 1) * P],
                                     start=False, stop=False)
                    return bv

                for c0, cols, pieces in calls_lo:
                    gt = gath.tile([P, call_cols * P], bf16, tag="gt")
                    gather(gt[:, : cols * P], tlo,
                           wlo[:, 8 * c0 : 8 * (c0 + cols)], cols * P)
                    for b, o, d, first, last in pieces:
                        if first:
                            start_block(b)
                        bv = blk_view(b, "ps_blk", "psg_lo")
                        for i in range(d):
                            _, sp_ = flags(b, False, last and i == d - 1)
                            nc.tensor.matmul(
                                out=bv, lhsT=ident[:],
                                rhs=gt[:, (o + i) * P : (o + i + 1) * P],
                                start=False, stop=sp_)
                        if last and b == g_last(b):
                            g = b // GB
                            gt_ps = grp.pop(g)
                            for r in range(b % GB + 1):
                                bb = g * GB + r
                                sl = slice(bb * P, (bb + 1) * P)
                                if layer == 2:
                                    nc.vector.tensor_scalar_mul(
                                        H[:, sl],
                                        gt_ps[:, r * P : (r + 1) * P],
                                        sarr[:, bb : bb + 1])
                                    continue
                                # layer 1: U = s*(agg); h1 = U @ W1
                                ub = stream.tile([P, P], bf16, tag="ub")
                                nc.vector.tensor_scalar_mul(
                                    ub[:], gt_ps[:, r * P : (r + 1) * P],
                                    sarr[:, bb : bb + 1])
                                psT = psp.tile([P, P], bf16, space="PSUM",
                                               tag="ps_bigT")
                                nc.tensor.transpose(out=psT[:], in_=ub[:],
                                                    identity=ident[:])
                                uT = stream.tile([P, P], bf16, tag="uT")
                                nc.vector.tensor_copy(out=uT[:], in_=psT[:])
                                ps1 = psp.tile([P, P], f32, space="PSUM",
                                               tag="ps_big")
                                nc.tensor.matmul(out=ps1[:], lhsT=uT[:],
                                                 rhs=w1[:], start=True,
                                                 stop=True)
                                nc.vector.tensor_copy(out=H[:, sl],
                                                      in_=ps1[:])
                                sq = stream.tile([P, D_HID], bf16,
                                                 tag="sq")
                                nc.scalar.square(out=sq[:], in_=H[:, sl])
                                nc.tensor.matmul(
                                    out=ps_stats[:, 0:1], lhsT=H[:, sl],
                                    rhs=ones_col[:],
                                    start=(bb == 0), stop=False)
                                nc.tensor.matmul(
                                    out=ps_stats[:, 1:2], lhsT=sq[:],
                                    rhs=ones_col[:],
                                    start=False,
                                    stop=(bb == blocks - 1))

            aggregate(t_xslo, t_xshi, acc1, ownslab, layer=1)

            # --- BN ---------------------------------------------------------
            st = small.tile([D_HID, 2], f32, tag="st")
            nc.vector.tensor_copy(out=st[:], in_=ps_stats[:])
            nc.sync.dma_start(out=st_in[:], in_=st[:])
            nc.gpsimd.collective_compute(
                "AllGather", mybir.AluOpType.bypass, replica_groups=groups,
                ins=[st_in[:]], outs=[st_out[:]],
            )
            st8 = small.tile([D_HID, 2 * N_CORES], f32, tag="st8")
            nc.sync.dma_start(
                out=st8.rearrange("p (c t) -> p c t", t=2),
                in_=st_out[:].rearrange("(c p t) -> p c t", p=D_HID, t=2),
            )
            st2 = small.tile([D_HID, 2], f32, tag="st2")
            nc.vector.reduce_sum(
                out=st2[:],
                in_=st8.rearrange("p (c t) -> p t c", t=2),
                axis=mybir.AxisListType.X)

            eps_col = small.tile([D_HID, 1], f32, tag="eps_col")
            nc.vector.memset(eps_col[:], BN_EPS)
            mean = small.tile([D_HID, 1], f32, tag="mean")
            msq = small.tile([D_HID, 1], f32, tag="msq")
            var = small.tile([D_HID, 1], f32, tag="var")
            std = small.tile([D_HID, 1], f32, tag="std")
            istd = small.tile([D_HID, 1], f32, tag="istd")
            gp = small.tile([D_HID, 1], f32, tag="gp")
            bp_ = small.tile([D_HID, 1], f32, tag="bp")
            nc.vector.tensor_scalar_mul(mean[:], st2[:, 0:1], inv_n)
            nc.vector.tensor_scalar_mul(msq[:], st2[:, 1:2], inv_n)
            nc.scalar.square(out=var[:], in_=mean[:])
            nc.vector.tensor_tensor(out=var[:], in0=msq[:], in1=var[:],
                                    op=mybir.AluOpType.subtract)
            nc.scalar.activation(out=std[:], in_=var[:],
                                 func=mybir.ActivationFunctionType.Sqrt,
                                 bias=eps_col[:])
            nc.vector.reciprocal(out=istd[:], in_=std[:])
            nc.vector.tensor_tensor(out=gp[:], in0=gcol[:], in1=istd[:],
                                    op=mybir.AluOpType.mult)
            nc.vector.tensor_tensor(out=bp_[:], in0=mean[:], in1=gp[:],
                                    op=mybir.AluOpType.mult)
            nc.vector.tensor_tensor(out=bp_[:], in0=bcol[:], in1=bp_[:],
                                    op=mybir.AluOpType.subtract)
            outer_bcast(gp[:], grep)
            outer_bcast(bp_[:], brep)

            grep4 = small.tile([P, 4 * P], bf16, tag="grep4")
            brep4 = small.tile([P, 4 * P], bf16, tag="brep4")
            for r in range(4):
                nc.vector.tensor_copy(out=grep4[:, r * P : (r + 1) * P],
                                      in_=grep[:])
                nc.vector.tensor_copy(out=brep4[:, r * P : (r + 1) * P],
                                      in_=brep[:])
            for b0 in range(0, blocks, 4):
                b1 = min(b0 + 4, blocks)
                w = (b1 - b0) * P
                sl4 = slice(b0 * P, b1 * P)
                t1 = stream.tile([P, 4 * P], bf16, tag="bn1")
                nc.vector.tensor_tensor(out=t1[:, :w], in0=H[:, sl4],
                                        in1=grep4[:, :w],
                                        op=mybir.AluOpType.mult)
                nc.vector.tensor_tensor(out=t1[:, :w], in0=t1[:, :w],
                                        in1=brep4[:, :w],
                                        op=mybir.AluOpType.add)
                nc.scalar.activation(out=t1[:, :w], in_=t1[:, :w],
                                     func=mybir.ActivationFunctionType.Relu)
                for b in range(b0, b1):
                    r = b - b0
                    nc.vector.tensor_scalar_mul(
                        ag2sb[:, b * P : (b + 1) * P],
                        t1[:, r * P : (r + 1) * P], sarr[:, b : b + 1])
                nc.sync.dma_start(out=ag2_in[:, b0 * P : b1 * P],
                                  in_=ag2sb[:, b0 * P : b1 * P])

            nc.gpsimd.collective_compute(
                "AllGather", mybir.AluOpType.bypass, replica_groups=groups,
                ins=[ag2_in[:]], outs=[tab2[:]],
            )

            aggregate(tab2[: LO_CORES * P, :],
                      tab2[LO_CORES * P :, :], acc2, ag2sb,
                      layer=2)

            for b in range(blocks):
                sl = slice(b * P, (b + 1) * P)
                pst = psp.tile([P, P], bf16, space="PSUM", tag="ps_bigT")
                nc.tensor.transpose(out=pst[:], in_=H[:, sl],
                                    identity=ident[:])
                uT = stream.tile([P, P], bf16, tag="uT")
                nc.vector.tensor_copy(out=uT[:], in_=pst[:])
                pso = psp.tile([P, P], f32, space="PSUM", tag="ps_big")
                nc.tensor.matmul(out=pso[:], lhsT=uT[:], rhs=wcat[:],
                                 start=True, stop=True)
                nc.vector.tensor_add(out=outsb[:, sl], in0=pso[:],
                                     in1=bcrep[:])
                if b % 8 == 7 or b == blocks - 1:
                    b0o = (b // 8) * 8
                    nc.sync.dma_start(
                        out=t_out[:, b0o * P : (b + 1) * P],
                        in_=outsb[:, b0o * P : (b + 1) * P])

    nc.compile()
    return nc


# ----------------------------------------------------------------------------
# Entry point
# ----------------------------------------------------------------------------

_IN_NAMES = ["xslo", "xshi", "xso", "W1", "Wcat", "bcat", "s_arr",
             "widx_lo", "widx_hi", "widx_cb", "gamma", "beta"]


def _geom(plan, call_cols):
    return (
        plan["npc"],
        plan["blocks"],
        tuple(int(d) for d in plan["D_lo"]),
        tuple(plan["calls_lo"]),
        plan["ct_lo"],
        tuple(int(d) for d in plan["D_hi"]),
        tuple(plan["calls_hi"]),
        plan["ct_hi"],
        int(plan["node_of"].max()) + 1,
        call_cols,
        plan["lo_rows"],
    )


def _run_hw(nc, per_core, trace=False, trace_cores=None):
    from concourse import bass_utils

    in_maps = [{nm: per_core[k][nm] for nm in _IN_NAMES} for k in range(N_CORES)]
    res = bass_utils.run_bass_kernel_spmd(
        nc, in_maps, core_ids=list(range(N_CORES)), trace=trace,
        trace_cores=trace_cores,
    )
    outs = [res.results[k]["out_cat"] for k in range(N_CORES)]
    return outs, res


def kernel(x, edge_index, W1, b1, gamma, beta, Wmu, bmu, Wls, bls):
    x = np.asarray(x, dtype=np.float32)
    edge_index = np.asarray(edge_index)
    W1 = np.asarray(W1, dtype=np.float32)
    gamma = np.asarray(gamma, dtype=np.float32)
    beta = np.asarray(beta, dtype=np.float32)
    Wmu = np.asarray(Wmu, dtype=np.float32)
    bmu = np.asarray(bmu, dtype=np.float32)
    Wls = np.asarray(Wls, dtype=np.float32)
    bls = np.asarray(bls, dtype=np.float32)

    plan = _plan(edge_index, x.shape[0], N_CORES, call_cols=CALL_COLS)
    per_core = _host_inputs(plan, x, W1, Wmu, Wls, bmu, bls, gamma, beta)

    geom = _geom(plan, CALL_COLS)
    if geom not in _CACHE:
        _CACHE[geom] = _build_program(geom)
    nc = _CACHE[geom]

    outs, _ = _run_hw(nc, per_core, trace=False)
    mu, ls = _postprocess(plan, outs)
    return mu, ls


# revision 28
# speedup vs baseline: 1.1555x; 1.0162x over previous
"""GCN encoder (2x GCNConv + BatchNorm/ReLU) on 8 Trainium2 NeuronCores.

Math: with s = 1/sqrt(deg+1) (deg = in-degree by dst), the GCN edge norm
factorizes: norm_e = s[src]*s[dst], so for any node features H,
    A(H) := segsum(norm_e * H[src], dst) + H * s^2
          = s * ( segsum( (s*H)[src], dst) + (s*H) )
and GCNConv(H, W, b) = A(H)@W + b = A(H@W) + b, so the whole net needs only
TWO sparse aggregations (layer1 on (s*x)@W1, layer2 on s*post-BN hidden),
and mu / log_std share the second one.

Design (vs the AllGather-everything baseline):
  * By linearity A(x@W1) == A(x)@W1, so layer 1 gathers RAW (s*x) messages:
    the host-marshaled bf16 x tables (replicated to every core's DRAM as
    inputs) ARE the gather tables -- no stage-1 compute, no table write, no
    AllGather #1 (saves a ~250us collective plus ~90us of DMA round trip).
    W1 is applied per block AFTER aggregation (transpose + 128x128 matmul),
    the same pattern the output stage uses for Wmu/Wls.
  * Message tables are bf16 (halves collective + HBM gather traffic).
  * Segment sums run on the PE: the gather layout puts slot-aligned edges on
    partitions, so each 128-column slot is accumulated into a per-block PSUM
    region with an identity-matmul (PSUM fp32 accumulate; 4 blocks share one
    2KB bank as a single accumulation group) -- DVE only does one scale per
    block.  Padding slots point at guaranteed-zero table rows.
  * All tables are 2-D partition-major ([core*128+p, block*128+f]), so every
    table/accumulator DMA is a plain contiguous slice at full line rate (no
    256B-descriptor penalty), and lo/hi halves are contiguous row ranges.
  * Gather call size stays at 7 cols * 128 = 896 indices (the hw SWDGE
    descriptor ring holds 128 in-flight entries per engine and a call needs
    num_idxs*2/16+1, so num_idxs <= ~1000; larger calls hang the device).
    The calls are off the critical path -- PE streaming and the AllGather
    bound the sections -- so the small call size costs nothing.
  * Self-loop term is added from an SBUF-resident copy of the core's own
    table slab (one plain DMA from the per-core x-slab input) via one more
    identity-matmul per block.
  * BN batch stats go through a tiny AllGather (cheaper than AllReduce) and
    are combined locally.  Only layer 2 still needs a real table AllGather.

Host preprocessing (free): integer edge plan, permutations, index tensors,
plus input marshaling: x is row-scaled by s, permuted to table order, and
cast to bf16 ((diag(s) x) @ W1 == s * (x @ W1)).  All O(N*D^2) and per-edge
float math stays on device.
"""

import numpy as np

N_NODES = 50000
N_EDGES = 800000
D_IN = 128
D_HID = 128
D_LAT = 64
BN_EPS = 1e-5
N_CORES = 8
P = 128
LO_CORES = 5  # cores 0..4 form the "lo" table half; 5*6272=31360 < 32768
              # (dma_gather int16 indices address at most 32768 rows per call)

CALL_COLS = 7      # gather call size: 7 cols * 128 = 896 idxs (hw ring cap)
NUM_Q = 4          # SWDGE queues

_CACHE = {}


# ----------------------------------------------------------------------------
# Host-side preprocessing
# ----------------------------------------------------------------------------


def _wrap_idx(lin):
    """dma_gather idx layout: position i -> [i%16, i//16], replicated to 128
    partitions. lin: [n] int array (n % 16 == 0) -> [128, n//16] int16."""
    n = lin.shape[0]
    w = lin.reshape(n // 16, 16).T.astype(np.int16)  # [16, n//16]
    return np.tile(w, (8, 1))


def _pack_calls(D, call_cols):
    """Slice the global column space into calls of <= call_cols columns.

    A call may cover partial blocks; each call carries its piece list
    [(block, col_off_in_call, width, first, last)].
    """
    C0 = np.concatenate([[0], np.cumsum(D)]).astype(np.int64)
    ct = int(C0[-1])
    calls = []
    for c0 in range(0, ct, call_cols):
        c1 = min(c0 + call_cols, ct)
        pieces = []
        for b in range(len(D)):
            lo = max(c0, int(C0[b]))
            hi = min(c1, int(C0[b + 1]))
            if lo < hi:
                pieces.append(
                    (b, lo - c0, hi - lo, lo == int(C0[b]), hi == int(C0[b + 1]))
                )
        calls.append((c0, c1 - c0, tuple(pieces)))
    return C0, calls


def _build_pass(tcoord_src, tkey_dst, n_cores, npc, blocks, call_cols,
                pad_idx, idx_base):
    """Build one gather pass layout.

    tcoord_src: per-edge source table coord (already offset for hi pass)
    tkey_dst:   per-edge dst node key in THIS pass's permutation
    Returns D [blocks], C0, calls, idx arrays [n_cores, 128, c_total] int32.
    """
    deg = np.bincount(tkey_dst, minlength=n_cores * npc)
    d3 = deg.reshape(n_cores, blocks, P)
    D = d3.max(axis=(0, 2)).astype(np.int64)
    D = np.maximum(D, 1)
    C0, calls = _pack_calls(D, call_cols)
    c_total = int(C0[-1])

    idx = np.full((n_cores, P, c_total), pad_idx - idx_base, dtype=np.int32)
    eorder = np.argsort(tkey_dst, kind="stable")
    k_s = tkey_dst[eorder]
    src_s = (tcoord_src[eorder] - idx_base).astype(np.int32)
    grp = np.searchsorted(k_s, k_s)
    slot = np.arange(k_s.size) - grp
    core_e = k_s // npc
    local_e = k_s % npc
    b_e = local_e // P
    p_e = local_e % P
    col_e = C0[b_e] + slot
    assert (slot < D[b_e]).all()
    idx[core_e, p_e, col_e] = src_s
    return D, C0, calls, idx, c_total


def _idx_to_wrapped(idx):
    """[n_cores, 128, c_total] int32 -> wrapped int16 [n_cores, 128, 8*c_total].

    Global linear position order is column-major (i = c*128 + p); contiguous
    position chunks map to contiguous wrapped columns, so any call covering
    cols [c0, c1) reads the wrapped slice [:, 8*c0 : 8*c1]."""
    n_cores, _, c_total = idx.shape
    out = np.empty((n_cores, 128, 8 * c_total), dtype=np.int16)
    for k in range(n_cores):
        lin = idx[k].T.reshape(-1)
        out[k] = _wrap_idx(lin)
    return out


def _plan(edge_index, n_nodes, n_cores, call_cols):
    src = np.asarray(edge_index[0], dtype=np.int64)
    dst = np.asarray(edge_index[1], dtype=np.int64)

    deg_in = np.bincount(dst, minlength=n_nodes).astype(np.int64)
    s = (1.0 / np.sqrt((deg_in + 1).astype(np.float64))).astype(np.float32)

    n_local = (n_nodes + n_cores - 1) // n_cores
    blocks = (n_local + 1 + P - 1) // P
    npc = blocks * P
    lo_rows = LO_CORES * npc
    assert lo_rows < 32768 and (n_cores * npc - lo_rows) < 32768

    # ---- core assignment: deal by total-degree rank (balances edge load and
    # aligns block-degree profiles across cores).
    order = np.argsort(-deg_in, kind="stable")
    rank_of = np.empty(n_nodes, dtype=np.int64)
    rank_of[order] = np.arange(n_nodes)
    core_of = rank_of % n_cores

    src_is_lo = core_of[src] < LO_CORES
    dlo = np.bincount(dst[src_is_lo], minlength=n_nodes)
    dhi = np.bincount(dst[~src_is_lo], minlength=n_nodes)

    # main layout: per-core locals sorted by lo-degree (tight LO padding)
    local_of = np.empty(n_nodes, dtype=np.int64)
    node2hi = np.empty(n_nodes, dtype=np.int64)
    for k in range(n_cores):
        nodes_k = np.nonzero(core_of == k)[0]
        o = nodes_k[np.argsort(-dlo[nodes_k], kind="stable")]
        local_of[o] = np.arange(o.size)
        o2 = nodes_k[np.argsort(-dhi[nodes_k], kind="stable")]
        node2hi[o2] = k * npc + np.arange(o2.size)
    node2table = core_of * npc + local_of


    # ---- gather-source row numbering: tables are [(core,part), (block,feat)]
    # 2-D tensors, so node (core k, local j=b*128+p) lives at flat row
    # (k*128+p)*blocks + b of its half (hi half: k-LO_CORES).
    def kpb_row(core, local, core0):
        return ((core - core0) * P + local % P) * blocks + local // P

    node2row = np.where(
        core_of < LO_CORES,
        kpb_row(core_of, local_of, 0),
        kpb_row(core_of, local_of, LO_CORES),
    )
    pad_lo = kpb_row(0, npc - 1, 0)
    pad_hi = kpb_row(N_CORES - 1, npc - 1, LO_CORES)

    # ---- LO pass on the main permutation
    D_lo, C0_lo, calls_lo, idx_lo, ct_lo = _build_pass(
        node2row[src[src_is_lo]], node2table[dst[src_is_lo]], n_cores, npc,
        blocks, call_cols, pad_lo, 0,
    )

    # ---- HI pass on the hi permutation
    D_hi, C0_hi, calls_hi, idx_hi, ct_hi = _build_pass(
        node2row[src[~src_is_lo]], node2hi[dst[~src_is_lo]], n_cores, npc,
        blocks, call_cols, pad_hi, 0,
    )

    # ---- combine map: main-layout local j gets acc_hi[himap[j]] added
    himap = np.full((n_cores, npc), npc - 1, dtype=np.int64)  # pad -> pad row
    for k in range(n_cores):
        nodes_k = np.nonzero(core_of == k)[0]
        himap[k, local_of[nodes_k]] = node2hi[nodes_k] % npc

    widx_lo = _idx_to_wrapped(idx_lo)
    widx_hi = _idx_to_wrapped(idx_hi)
    himap_row = (himap % P) * blocks + himap // P
    widx_cb = np.stack([_wrap_idx(himap_row[k]) for k in range(n_cores)])

    # per-core node lists and s in the MAIN layout
    node_of = np.full((n_cores, npc), -1, dtype=np.int64)
    s_arr = np.zeros((n_cores, P, blocks), dtype=np.float32)
    for k in range(n_cores):
        nodes_k = np.nonzero(core_of == k)[0]
        loc = local_of[nodes_k]
        node_of[k, loc] = nodes_k
        s_arr[k, loc % P, loc // P] = s[nodes_k]

    return dict(
        s=s,
        node2table=node2table,
        node_of=node_of,
        npc=npc,
        blocks=blocks,
        n_local=n_local,
        lo_rows=lo_rows,
        s_arr=s_arr,
        D_lo=D_lo, C0_lo=C0_lo, calls_lo=calls_lo, idx_lo=idx_lo, ct_lo=ct_lo,
        D_hi=D_hi, C0_hi=C0_hi, calls_hi=calls_hi, idx_hi=idx_hi, ct_hi=ct_hi,
        himap=himap,
        widx_lo=widx_lo, widx_hi=widx_hi, widx_cb=widx_cb,
    )


def _to_bf16(a):
    import ml_dtypes

    return a.astype(ml_dtypes.bfloat16)


def _host_inputs(plan, x, W1, Wmu, Wls, bmu, bls, gamma, beta):
    npc = plan["npc"]
    node_of = plan["node_of"]
    s = plan["s"]
    wcat = np.concatenate([Wmu, Wls], axis=1).astype(np.float32)
    bcat = np.concatenate([bmu, bls]).astype(np.float32).reshape(D_HID, 1)

    # s-prescaled x in the partition-major table layout shared with tab2:
    # row (core*128 + p), col (block*128 + f) holds node (core, b*128+p).
    blocks = npc // P
    xs_tab = np.zeros((N_CORES * npc, x.shape[1]), dtype=np.float32)
    for k in range(N_CORES):
        nodes = node_of[k]
        valid = nodes >= 0
        xs_tab[k * npc + np.nonzero(valid)[0]] = (
            x[nodes[valid]] * s[nodes[valid]][:, None]
        )
    xs_kpb = (
        xs_tab.reshape(N_CORES, blocks, P, D_IN)
        .transpose(0, 2, 1, 3)
        .reshape(N_CORES * P, blocks * D_IN)
    )
    xs_kpb = _to_bf16(xs_kpb)
    xs_lo = np.ascontiguousarray(xs_kpb[: LO_CORES * P])
    xs_hi = np.ascontiguousarray(xs_kpb[LO_CORES * P :])

    per_core = []
    for k in range(N_CORES):
        per_core.append(
            {
                "xslo": xs_lo,
                "xshi": xs_hi,
                "xso": np.ascontiguousarray(xs_kpb[k * P : (k + 1) * P]),
                "W1": np.ascontiguousarray(_to_bf16(W1)),
                "Wcat": np.ascontiguousarray(_to_bf16(wcat)),
                "bcat": bcat,
                "s_arr": np.ascontiguousarray(plan["s_arr"][k]),
                "widx_lo": np.ascontiguousarray(plan["widx_lo"][k]),
                "widx_hi": np.ascontiguousarray(plan["widx_hi"][k]),
                "widx_cb": np.ascontiguousarray(plan["widx_cb"][k]),
                "gamma": gamma.astype(np.float32).reshape(D_HID, 1),
                "beta": beta.astype(np.float32).reshape(D_HID, 1),
            }
        )
    return per_core


def _postprocess(plan, outs):
    n_nodes = int(plan["node_of"].max()) + 1
    mu = np.zeros((n_nodes, D_LAT), dtype=np.float32)
    ls = np.zeros((n_nodes, D_LAT), dtype=np.float32)
    node_of = plan["node_of"]
    npc = node_of.shape[1]
    blocks = npc // 128
    for k in range(N_CORES):
        nodes = node_of[k]
        valid = nodes >= 0
        o = np.asarray(outs[k]).astype(np.float32).reshape(128, blocks, 128)
        o = o.transpose(1, 0, 2).reshape(npc, 128)  # node-major (b*128+p)
        mu[nodes[valid]] = o[valid.nonzero()[0], :D_LAT]
        ls[nodes[valid]] = o[valid.nonzero()[0], D_LAT:]
    return mu, ls


# ----------------------------------------------------------------------------
# Numpy emulation of the device program (fp32; bf16 rounding not modeled)
# ----------------------------------------------------------------------------


def _numpy_model(plan, per_core, x, W1, wcat, bcat, gamma, beta, n_real):
    npc, blocks = plan["npc"], plan["blocks"]
    lo_rows = plan["lo_rows"]
    node_of, s = plan["node_of"], plan["s"]

    table1 = np.zeros((N_CORES * npc, D_HID), dtype=np.float32)
    for k in range(N_CORES):
        nodes = node_of[k]
        valid = nodes >= 0
        table1[k * npc + np.nonzero(valid)[0]] = (
            (x[nodes[valid]] * s[nodes[valid]][:, None]) @ W1
        )

    def aggregate(table, k):
        out = np.zeros((npc, table.shape[1]), dtype=np.float32)
        mlo = table[:lo_rows][plan["idx_lo"][k]]  # [P, ct, F]
        C0 = plan["C0_lo"]
        for b in range(blocks):
            out[b * P:(b + 1) * P] += mlo[:, C0[b]:C0[b + 1], :].sum(axis=1)
        mhi = table[lo_rows:][plan["idx_hi"][k]]
        acc_hi = np.zeros((npc, table.shape[1]), dtype=np.float32)
        C0 = plan["C0_hi"]
        for b in range(blocks):
            acc_hi[b * P:(b + 1) * P] = mhi[:, C0[b]:C0[b + 1], :].sum(axis=1)
        out += acc_hi[plan["himap"][k]]
        own0 = k * npc
        sk = per_core[k]["s_arr"].T.reshape(-1, 1)
        return (out + table[own0:own0 + npc]) * sk

    h1 = [aggregate(table1, k) for k in range(N_CORES)]
    allh1 = np.concatenate(h1, axis=0)
    mean = allh1.sum(axis=0) / n_real
    var = (allh1 * allh1).sum(axis=0) / n_real - mean * mean
    inv = 1.0 / np.sqrt(var + BN_EPS)
    g2 = gamma * inv
    b2 = beta - mean * g2

    table2 = np.zeros((N_CORES * npc, D_HID), dtype=np.float32)
    for k in range(N_CORES):
        sk = per_core[k]["s_arr"].T.reshape(-1, 1)
        h2 = np.maximum(h1[k] * g2[None, :] + b2[None, :], 0.0)
        table2[k * npc:(k + 1) * npc] = h2 * sk

    outs = []
    for k in range(N_CORES):
        u = aggregate(table2, k)
        outs.append((u @ wcat + bcat.reshape(1, -1)).astype(np.float32))
    return outs


# ----------------------------------------------------------------------------
# Device program
# ----------------------------------------------------------------------------


def _build_program(geom):
    from concourse import bacc, bass, mybir, tile
    from concourse.masks import make_identity

    (npc, blocks, D_lo, calls_lo, ct_lo, D_hi, calls_hi, ct_hi, n_real,
     call_cols, lo_rows) = geom
    D_lo, D_hi = list(D_lo), list(D_hi)
    f32 = mybir.dt.float32
    bf16 = mybir.dt.bfloat16
    i16 = mybir.dt.int16
    n_tab = N_CORES * npc

    nc = bacc.Bacc("TRN2", target_bir_lowering=False, debug=False,
                   num_devices=N_CORES, num_swdge_queues=NUM_Q)

    t_xslo = nc.dram_tensor("xslo", [LO_CORES * P, npc], bf16,
                            kind="ExternalInput")
    t_xshi = nc.dram_tensor("xshi", [(N_CORES - LO_CORES) * P, npc], bf16,
                            kind="ExternalInput")
    t_xso = nc.dram_tensor("xso", [P, npc], bf16, kind="ExternalInput")
    t_W1 = nc.dram_tensor("W1", [P, D_HID], bf16, kind="ExternalInput")
    t_Wcat = nc.dram_tensor("Wcat", [D_HID, P], bf16, kind="ExternalInput")
    t_bcat = nc.dram_tensor("bcat", [P, 1], f32, kind="ExternalInput")
    t_sarr = nc.dram_tensor("s_arr", [P, blocks], f32, kind="ExternalInput")
    t_wlo = nc.dram_tensor("widx_lo", [P, 8 * ct_lo], i16, kind="ExternalInput")
    t_whi = nc.dram_tensor("widx_hi", [P, 8 * ct_hi], i16, kind="ExternalInput")
    t_wcb = nc.dram_tensor("widx_cb", [P, npc // 16], i16, kind="ExternalInput")
    t_gamma = nc.dram_tensor("gamma", [D_HID, 1], f32, kind="ExternalInput")
    t_beta = nc.dram_tensor("beta", [D_HID, 1], f32, kind="ExternalInput")
    t_out = nc.dram_tensor("out_cat", [P, npc], bf16, kind="ExternalOutput")

    ag2_in = nc.dram_tensor("ag2_in", [P, npc], bf16)
    tab2 = nc.dram_tensor("tab2", [N_CORES * P, npc], bf16,
                          addr_space="Shared")
    acc1 = nc.dram_tensor("acc_hi1", [P, npc], bf16)
    acc2 = nc.dram_tensor("acc_hi2", [P, npc], bf16)
    st_in = nc.dram_tensor("st_in", [2 * D_HID], f32)
    st_out = nc.dram_tensor("st_out", [N_CORES * 2 * D_HID], f32,
                            addr_space="Shared")

    groups = [list(range(N_CORES))]
    inv_n = 1.0 / float(n_real)

    with tile.TileContext(nc) as tc:
        with (
            tc.tile_pool(name="persist", bufs=1) as persist,
            tc.tile_pool(name="stream", bufs=4) as stream,
            tc.tile_pool(name="gath", bufs=3) as gath,
            tc.tile_pool(name="gathcb", bufs=1) as gathcb,
            tc.tile_pool(name="small", bufs=1) as small,
            tc.tile_pool(name="ps", bufs=2, space="PSUM") as psp,
            tc.tile_pool(name="ps_blk", bufs=3, space="PSUM") as psblk,
            tc.tile_pool(name="ps_acc", bufs=1, space="PSUM") as psacc,
        ):
            H = persist.tile([P, npc], bf16, tag="H")
            ownslab = persist.tile([P, npc], bf16, tag="ownslab")
            ag2sb = persist.tile([P, npc], bf16, tag="ag2sb")
            accsb = persist.tile([P, npc], bf16, tag="accsb")
            outsb = persist.tile([P, npc], bf16, tag="outsb")
            wlo = persist.tile([P, 8 * ct_lo], i16, tag="wlo")
            whi = persist.tile([P, 8 * ct_hi], i16, tag="whi")
            wcb = persist.tile([P, npc // 16], i16, tag="wcb")
            w1 = small.tile([P, D_HID], bf16, tag="w1")
            wcat = small.tile([D_HID, P], bf16, tag="wcat")
            sarr = small.tile([P, blocks], f32, tag="sarr")
            gcol = small.tile([D_HID, 1], f32, tag="gcol")
            bcol = small.tile([D_HID, 1], f32, tag="bcol")
            bccol = small.tile([P, 1], f32, tag="bccol")
            ident = small.tile([P, P], bf16, tag="ident")
            identf = small.tile([P, P], f32, tag="identf")
            ones_col = small.tile([P, 1], bf16, tag="ones_col")
            ones_row = small.tile([1, P], f32, tag="ones_row")
            grep = small.tile([P, P], f32, tag="grep")
            brep = small.tile([P, P], f32, tag="brep")
            bcrep = small.tile([P, P], f32, tag="bcrep")

            nc.sync.dma_start(out=whi[:], in_=t_whi[:])
            nc.sync.dma_start(out=wlo[:], in_=t_wlo[:])
            nc.sync.dma_start(out=wcb[:], in_=t_wcb[:])
            nc.sync.dma_start(out=w1[:], in_=t_W1[:])
            nc.sync.dma_start(out=wcat[:], in_=t_Wcat[:])
            nc.sync.dma_start(out=sarr[:], in_=t_sarr[:])
            nc.sync.dma_start(out=gcol[:], in_=t_gamma[:])
            nc.sync.dma_start(out=bcol[:], in_=t_beta[:])
            nc.sync.dma_start(out=bccol[:], in_=t_bcat[:])
            make_identity(nc, ident[:])
            make_identity(nc, identf[:])
            nc.vector.memset(ones_col[:], 1.0)
            nc.vector.memset(ones_row[:], 1.0)

            def outer_bcast(col_ap, dst_tile):
                pst = psp.tile([P, P], f32, space="PSUM", tag="ps_big")
                nc.tensor.transpose(out=pst[0:1, :], in_=col_ap,
                                    identity=identf[:])
                row = stream.tile([1, P], f32, tag="rowbuf")
                nc.vector.tensor_copy(out=row[:], in_=pst[0:1, :])
                psb = psp.tile([P, P], f32, space="PSUM", tag="ps_big")
                nc.tensor.matmul(out=psb[:], lhsT=ones_row[:], rhs=row[:],
                                 start=True, stop=True)
                nc.vector.tensor_copy(out=dst_tile[:], in_=psb[:])

            outer_bcast(bccol[:], bcrep)

            # --- layer 1 gathers raw (s*x): A(x@W1) == A(x)@W1, so W1 is
            # applied per-block AFTER aggregation; the host-marshaled xslo/
            # xshi inputs ARE the gather tables (no stage-1, no table write).
            nc.sync.dma_start(out=ownslab[:], in_=t_xso[:, :])

            ps_stats = psacc.tile([D_HID, 2], f32, space="PSUM",
                                  tag="ps_stats")

            qn = [0]

            def next_q():
                q = qn[0]
                qn[0] = (qn[0] + 1) % NUM_Q
                return q

            def gather(out_ap, table_ap, widx_ap, n_idx):
                nc.gpsimd.dma_gather(
                    out_ap.rearrange("p (c f) -> p c f", f=P),
                    table_ap.rearrange("q (r f) -> (q r) f", f=P),
                    widx_ap,
                    num_idxs=n_idx,
                    num_idxs_reg=n_idx,
                    elem_size=P,
                    queue_num=next_q(),
                )

            GB = 4  # blocks per PSUM supergroup

            def aggregate(tlo, thi, acc_t, selfslab, layer):
                # ---- HI pass: per-block PSUM sums (4 blocks/bank) -> accsb
                grp = {}

                def blk_view(b, tag, pre):
                    g, r = b // GB, b % GB
                    if g not in grp:
                        grp[g] = psblk.tile([P, GB * P], f32, space="PSUM",
                                            tag="ps_blk",
                                            name=f"{pre}_{layer}_{g}")
                    return grp[g][:, r * P : (r + 1) * P]

                g_open = set()

                def g_last(b):
                    return min((b // GB) * GB + GB - 1, blocks - 1)

                def flags(b, is_first_mm, is_last_mm):
                    g = b // GB
                    start = is_first_mm and g not in g_open
                    if start:
                        g_open.add(g)
                    stop = is_last_mm and b == g_last(b)
                    return start, stop

                for c0, cols, pieces in calls_hi:
                    gt = gath.tile([P, call_cols * P], bf16, tag="gt")
                    gather(gt[:, : cols * P], thi,
                           whi[:, 8 * c0 : 8 * (c0 + cols)], cols * P)
                    for b, o, d, first, last in pieces:
                        bv = blk_view(b, "ps_blk", "psg_hi")
                        for i in range(d):
                            st_, sp_ = flags(b, first and i == 0,
                                             last and i == d - 1)
                            nc.tensor.matmul(
                                out=bv, lhsT=ident[:],
                                rhs=gt[:, (o + i) * P : (o + i + 1) * P],
                                start=st_, stop=sp_)
                        if last and b == g_last(b):
                            g = b // GB
                            w = (b % GB) + 1
                            nc.vector.tensor_copy(
                                out=accsb[:, g * GB * P : g * GB * P + w * P],
                                in_=grp.pop(g)[:, : w * P])
                nc.sync.dma_start(out=acc_t[:, :], in_=accsb[:])

                # combine gathers (cbt[:, j] = acc_t[himap[j]]) are issued
                # a few calls INTO the lo stream (below) so the acc-write
                # latency hides behind lo gather work.
                cbt = gathcb.tile([P, npc], bf16, tag="gtcb",
                                  name=f"gtcb_{layer}")

                # ---- LO pass + self + cb into per-block PSUM, then scale
                grp.clear()
                g_open.clear()

                def start_block(b):
                    bv = blk_view(b, "ps_blk", "psg_lo")
                    st_, _ = flags(b, True, False)
                    nc.tensor.matmul(out=bv, lhsT=ident[:],
                                     rhs=selfslab[:, b * P : (b + 1) * P],
                                     start=st_, stop=False)
                    return bv

                # cb gathers go right after the first lo call: late enough
                # to overlap the acc write with lo work, but in program order
                # BEFORE any block's cb matmul (earliest block completion is
                # at lo call (D_lo[0]-1)//call_cols >= 1).
                cb_at = min(1, (D_lo[0] - 1) // call_cols)
                for ci, (c0, cols, pieces) in enumerate(calls_lo):
                    if ci == cb_at:
                        for cb0 in range(0, blocks, call_cols):
                            cb1 = min(cb0 + call_cols, blocks)
                            gather(cbt[:, cb0 * P : cb1 * P], acc_t,
                                   wcb[:, 8 * cb0 : 8 * cb1],
                                   (cb1 - cb0) * P)
                    gt = gath.tile([P, call_cols * P], bf16, tag="gt")
                    gather(gt[:, : cols * P], tlo,
                           wlo[:, 8 * c0 : 8 * (c0 + cols)], cols * P)
                    for b, o, d, first, last in pieces:
                        if first:
                            start_block(b)
                        bv = blk_view(b, "ps_blk", "psg_lo")
                        for i in range(d):
                            nc.tensor.matmul(
                                out=bv, lhsT=ident[:],
                                rhs=gt[:, (o + i) * P : (o + i + 1) * P],
                                start=False, stop=False)
                        if last:
                            _, sp_ = flags(b, False, True)
                            nc.tensor.matmul(
                                out=bv, lhsT=ident[:],
                                rhs=cbt[:, b * P : (b + 1) * P],
                                start=False, stop=sp_)
                        if last and b == g_last(b):
                            g = b // GB
                            gt_ps = grp.pop(g)
                            flushed = []
                            for r in range(b % GB + 1):
                                bb = g * GB + r
                                sl = slice(bb * P, (bb + 1) * P)
                                if layer == 2:
                                    ub2 = stream.tile([P, P], bf16, tag="ub")
                                    nc.vector.tensor_scalar_mul(
                                        ub2[:],
                                        gt_ps[:, r * P : (r + 1) * P],
                                        sarr[:, bb : bb + 1])
                                    psT2 = psp.tile([P, P], bf16,
                                                    space="PSUM",
                                                    tag="ps_bigT")
                                    nc.tensor.transpose(out=psT2[:],
                                                        in_=ub2[:],
                                                        identity=ident[:])
                                    uT2 = stream.tile([P, P], bf16, tag="uT")
                                    nc.vector.tensor_copy(out=uT2[:],
                                                          in_=psT2[:])
                                    pso2 = psp.tile([P, P], f32,
                                                    space="PSUM",
                                                    tag="ps_big")
                                    nc.tensor.matmul(out=pso2[:],
                                                     lhsT=uT2[:],
                                                     rhs=wcat[:],
                                                     start=True, stop=True)
                                    nc.vector.tensor_add(
                                        out=outsb[:, sl], in0=pso2[:],
                                        in1=bcrep[:])
                                    flushed.append(bb)
                                    continue
                                # layer 1: U = s*(agg); h1 = U @ W1
                                ub = stream.tile([P, P], bf16, tag="ub")
                                nc.vector.tensor_scalar_mul(
                                    ub[:], gt_ps[:, r * P : (r + 1) * P],
                                    sarr[:, bb : bb + 1])
                                psT = psp.tile([P, P], bf16, space="PSUM",
                                               tag="ps_bigT")
                                nc.tensor.transpose(out=psT[:], in_=ub[:],
                                                    identity=ident[:])
                                uT = stream.tile([P, P], bf16, tag="uT")
                                nc.vector.tensor_copy(out=uT[:], in_=psT[:])
                                ps1 = psp.tile([P, P], f32, space="PSUM",
                                               tag="ps_big")
                                nc.tensor.matmul(out=ps1[:], lhsT=uT[:],
                                                 rhs=w1[:], start=True,
                                                 stop=True)
                                nc.vector.tensor_copy(out=H[:, sl],
                                                      in_=ps1[:])
                                sq = stream.tile([P, D_HID], bf16,
                                                 tag="sq")
                                nc.scalar.square(out=sq[:], in_=H[:, sl])
                                nc.tensor.matmul(
                                    out=ps_stats[:, 0:1], lhsT=H[:, sl],
                                    rhs=ones_col[:],
                                    start=(bb == 0), stop=False)
                                nc.tensor.matmul(
                                    out=ps_stats[:, 1:2], lhsT=sq[:],
                                    rhs=ones_col[:],
                                    start=False,
                                    stop=(bb == blocks - 1))
                            if flushed:
                                fb0, fb1 = flushed[0], flushed[-1] + 1
                                nc.sync.dma_start(
                                    out=t_out[:, fb0 * P : fb1 * P],
                                    in_=outsb[:, fb0 * P : fb1 * P])

            aggregate(t_xslo, t_xshi, acc1, ownslab, layer=1)

            # --- BN ---------------------------------------------------------
            st = small.tile([D_HID, 2], f32, tag="st")
            nc.vector.tensor_copy(out=st[:], in_=ps_stats[:])
            nc.sync.dma_start(out=st_in[:], in_=st[:])
            nc.gpsimd.collective_compute(
                "AllGather", mybir.AluOpType.bypass, replica_groups=groups,
                ins=[st_in[:]], outs=[st_out[:]],
            )
            st8 = small.tile([D_HID, 2 * N_CORES], f32, tag="st8")
            nc.sync.dma_start(
                out=st8.rearrange("p (c t) -> p c t", t=2),
                in_=st_out[:].rearrange("(c p t) -> p c t", p=D_HID, t=2),
            )
            st2 = small.tile([D_HID, 2], f32, tag="st2")
            nc.vector.reduce_sum(
                out=st2[:],
                in_=st8.rearrange("p (c t) -> p t c", t=2),
                axis=mybir.AxisListType.X)

            eps_col = small.tile([D_HID, 1], f32, tag="eps_col")
            nc.vector.memset(eps_col[:], BN_EPS)
            mean = small.tile([D_HID, 1], f32, tag="mean")
            msq = small.tile([D_HID, 1], f32, tag="msq")
            var = small.tile([D_HID, 1], f32, tag="var")
            std = small.tile([D_HID, 1], f32, tag="std")
            istd = small.tile([D_HID, 1], f32, tag="istd")
            gp = small.tile([D_HID, 1], f32, tag="gp")
            bp_ = small.tile([D_HID, 1], f32, tag="bp")
            nc.vector.tensor_scalar_mul(mean[:], st2[:, 0:1], inv_n)
            nc.vector.tensor_scalar_mul(msq[:], st2[:, 1:2], inv_n)
            nc.scalar.square(out=var[:], in_=mean[:])
            nc.vector.tensor_tensor(out=var[:], in0=msq[:], in1=var[:],
                                    op=mybir.AluOpType.subtract)
            nc.scalar.activation(out=std[:], in_=var[:],
                                 func=mybir.ActivationFunctionType.Sqrt,
                                 bias=eps_col[:])
            nc.vector.reciprocal(out=istd[:], in_=std[:])
            nc.vector.tensor_tensor(out=gp[:], in0=gcol[:], in1=istd[:],
                                    op=mybir.AluOpType.mult)
            nc.vector.tensor_tensor(out=bp_[:], in0=mean[:], in1=gp[:],
                                    op=mybir.AluOpType.mult)
            nc.vector.tensor_tensor(out=bp_[:], in0=bcol[:], in1=bp_[:],
                                    op=mybir.AluOpType.subtract)
            outer_bcast(gp[:], grep)
            outer_bcast(bp_[:], brep)

            grep4 = small.tile([P, 4 * P], bf16, tag="grep4")
            brep4 = small.tile([P, 4 * P], bf16, tag="brep4")
            for r in range(4):
                nc.vector.tensor_copy(out=grep4[:, r * P : (r + 1) * P],
                                      in_=grep[:])
                nc.vector.tensor_copy(out=brep4[:, r * P : (r + 1) * P],
                                      in_=brep[:])
            for b0 in range(0, blocks, 4):
                b1 = min(b0 + 4, blocks)
                w = (b1 - b0) * P
                sl4 = slice(b0 * P, b1 * P)
                t1 = stream.tile([P, 4 * P], bf16, tag="bn1")
                nc.vector.tensor_tensor(out=t1[:, :w], in0=H[:, sl4],
                                        in1=grep4[:, :w],
                                        op=mybir.AluOpType.mult)
                nc.vector.tensor_tensor(out=t1[:, :w], in0=t1[:, :w],
                                        in1=brep4[:, :w],
                                        op=mybir.AluOpType.add)
                nc.scalar.activation(out=t1[:, :w], in_=t1[:, :w],
                                     func=mybir.ActivationFunctionType.Relu)
                for b in range(b0, b1):
                    r = b - b0
                    nc.vector.tensor_scalar_mul(
                        ag2sb[:, b * P : (b + 1) * P],
                        t1[:, r * P : (r + 1) * P], sarr[:, b : b + 1])
                nc.sync.dma_start(out=ag2_in[:, b0 * P : b1 * P],
                                  in_=ag2sb[:, b0 * P : b1 * P])

            nc.gpsimd.collective_compute(
                "AllGather", mybir.AluOpType.bypass, replica_groups=groups,
                ins=[ag2_in[:]], outs=[tab2[:]],
            )

            aggregate(tab2[: LO_CORES * P, :],
                      tab2[LO_CORES * P :, :], acc2, ag2sb,
                      layer=2)

    nc.compile()
    return nc


# ----------------------------------------------------------------------------
# Entry point
# ----------------------------------------------------------------------------

_IN_NAMES = ["xslo", "xshi", "xso", "W1", "Wcat", "bcat", "s_arr",
             "widx_lo", "widx_hi", "widx_cb", "gamma", "beta"]


def _geom(plan, call_cols):
    return (
        plan["npc"],
        plan["blocks"],
        tuple(int(d) for d in plan["D_lo"]),
        tuple(plan["calls_lo"]),
        plan["ct_lo"],
        tuple(int(d) for d in plan["D_hi"]),
        tuple(plan["calls_hi"]),
        plan["ct_hi"],
        int(plan["node_of"].max()) + 1,
        call_cols,
        plan["lo_rows"],
    )


def _run_hw(nc, per_core, trace=False, trace_cores=None):
    from concourse import bass_utils

    in_maps = [{nm: per_core[k][nm] for nm in _IN_NAMES} for k in range(N_CORES)]
    res = bass_utils.run_bass_kernel_spmd(
        nc, in_maps, core_ids=list(range(N_CORES)), trace=trace,
        trace_cores=trace_cores,
    )
    outs = [res.results[k]["out_cat"] for k in range(N_CORES)]
    return outs, res


def kernel(x, edge_index, W1, b1, gamma, beta, Wmu, bmu, Wls, bls):
    x = np.asarray(x, dtype=np.float32)
    edge_index = np.asarray(edge_index)
    W1 = np.asarray(W1, dtype=np.float32)
    gamma = np.asarray(gamma, dtype=np.float32)
    beta = np.asarray(beta, dtype=np.float32)
    Wmu = np.asarray(Wmu, dtype=np.float32)
    bmu = np.asarray(bmu, dtype=np.float32)
    Wls = np.asarray(Wls, dtype=np.float32)
    bls = np.asarray(bls, dtype=np.float32)

    plan = _plan(edge_index, x.shape[0], N_CORES, call_cols=CALL_COLS)
    per_core = _host_inputs(plan, x, W1, Wmu, Wls, bmu, bls, gamma, beta)

    geom = _geom(plan, CALL_COLS)
    if geom not in _CACHE:
        _CACHE[geom] = _build_program(geom)
    nc = _CACHE[geom]

    outs, _ = _run_hw(nc, per_core, trace=False)
    mu, ls = _postprocess(plan, outs)
    return mu, ls
